# revision 59
# baseline (speedup 1.0000x reference)
"""Trainium2 Bass kernel for nn_ExpressionModel (dense DiT-style transformer block).

Sharding: 8 cores = 2 (batch) x 4 (sequence chunks of 512 tokens).
Each core computes the full block for its 512 query tokens; K/V projections
for the full 2048-token batch are duplicated across the 4 cores of a batch
(no collectives needed).

Key layout trick (vs the previous revision): q/k projection output tiles are
(head-group hg of 4 heads, j) with j in {0,1} the rotate-half block of the
head dim; partition p = (h%4)*32 + d%32. The rope "swap" partner of
partition p in block j is partition p in block 1-j of the SAME tile pair --
no swapped-weight second projection and no partition shuffles. The j dim
doubles as the fp8 DoubleRow pairing dim, so:
  - scores run fp8-DR [32,2]-stationary (0.5 cyc/col vs 1.0 bf16). PE
    operand base partitions must be 0/32/64, so the 4th head of each group
    reads a base-64 [64,2] window against a shadow q tile (qz8/qcz8) whose
    sibling-head rows are zeroed.
  - the exp writes probs as fp8 [128, 2, LQ] and p@V runs one fp8-DR
    matmul per 256-key window (vsb [128, 2, H, D+1]) -- 4x cheaper than
    bf16 128-key chunks.
Engine budget: Act's exps are the hard floor (~1038ns per [128,1024]; SA
133us + CA 33us); everything else is spread over DVE/Pool/Act by measured
cost (Pool cannot touch PSUM, scalar_tensor_tensor is DVE-only). DMA is a
single serial ~360GB/s resource, so the preamble queue order is arrival-
priority: x0, consts, adaLN-A (fp8), cos/sin, w_q, x1, x2, w_k, x3, w_v.
k head-groups 2-3 and the audio cross-K/V stream inside the self-attn
exp stream; adaLN-B columns stream weight-stationary the same way.
The MLP keeps the hi+lo fp8 split (T ~ T_hi + T_lo/64) on gate/up (3-pass)
and down (2-pass) -- measured HW rel err 0.0138 of the 0.02 budget; 2-pass
gate/up variants model out to ~0.019 on HW, too close to ship. Silu is a
single Act op (table includes copy/identity so no thrash with h64 scaling).
"""

import numpy as np
import ml_dtypes

import concourse.bass as bass
import concourse.tile as tile
from concourse import bacc, mybir
from concourse.bass_utils import run_bass_kernel_spmd

FP32 = mybir.dt.float32
BF16 = mybir.dt.bfloat16
F8 = mybir.dt.float8e4
DR = mybir.MatmulPerfMode.DoubleRow
F8NP = ml_dtypes.float8_e4m3

STAGE_MARKS = []  # (instruction-id watermark, stage name) — profiling aid

B, L, C = 2, 2048, 1024
H, D = 16, 64
L2, TD = 512, 768
FF = 4096
EPS = 1e-6
NCORE = 8
LQ = 512            # query tokens per core
CT = C // 128       # 8 C partition-tiles
KP = C // 256       # 4 DoubleRow contraction pairs over C
LCH = L // 512      # 4 512-token chunks
KSC = 1.0 / 8.0     # 1/sqrt(D)
LOSC = 64.0         # hi/lo split scale
NW = L // 256       # 8 256-key windows (self attn)
NWC = L2 // 256     # 2 windows (cross attn)

AF = mybir.ActivationFunctionType


def build_bass():
    nc = bacc.Bacc("TRN2", target_bir_lowering=False, debug=False)
    STAGE_MARKS.clear()

    def mark(stage):
        STAGE_MARKS.append((nc.next_id(), stage))

    def dma(out, in_):
        return nc.sync.dma_start(out=out, in_=in_)

    def din(name, shape, dt):
        return nc.dram_tensor(name, list(shape), dt, kind="ExternalInput")

    # --- inputs ---
    x_bf = din("x_bf", (C, L), BF16)            # x[b].T, bf16
    xq_f = din("xq_f", (C, LQ), FP32)           # own-chunk x[b].T, fp32 residual
    aud2 = din("aud2", (128, 3, 2, L2), F8)     # audio.T fp8 DR-paired
    cst = din("cst", (128, 80), FP32)           # tmod|adab|n1|n2|n3
    cs4 = din("cs4", (128, L), BF16)            # cos[p%32] rows
    ss4 = din("ss4", (128, L), BF16)            # sin[p%32] rows (plain)
    wadaA = din("wadaA", (128, CT, 2048), F8)      # adaLN W cols j0..15
    wadaB = din("wadaB", (8, 128, CT, 512), BF16)  # adaLN W cols j16..47
    wq2 = din("wq2", (128, KP, 2, C), F8)       # W_qkv q, (hg,j) DR layout
    wk2 = din("wk2", (128, KP, 2, C), F8)
    wv2 = din("wv2", (128, KP, 2, C), F8)       # v natural
    wsa2 = din("wsa2", (128, KP, 2, C), F8)
    wqc2 = din("wqc2", (128, KP, 2, C), F8)     # cross q, (hg,j) layout
    wkv2 = din("wkv2", (128, 3, 2, 2 * C), F8)  # K half (hg,j), V natural
    wca2 = din("wca2", (128, KP, 2, C), F8)
    wgh = din("wgh", (8, 128, KP, 2, 512), F8)  # MLP weights hi/lo fp8
    wgl = din("wgl", (8, 128, KP, 2, 512), F8)
    wuh = din("wuh", (8, 128, KP, 2, 512), F8)
    wul = din("wul", (8, 128, KP, 2, 512), F8)
    wdh = din("wdh", (CT, 128, 16, 2, 128), F8)
    wdl = din("wdl", (CT, 128, 16, 2, 128), F8)

    outT = nc.dram_tensor("outT", [C, LQ], FP32, kind="ExternalOutput")

    with tile.TileContext(nc) as tc:
        with (
            tc.tile_pool(name="pp", bufs=1) as pp,              # persistent
            tc.tile_pool(name="ps", bufs=1, space="PSUM") as ps,
        ):
            # ---- persistent constants (one packed tile) ----
            c_all = pp.tile([128, 80], FP32, tag="c_all")
            c_tmod = c_all[:, 0:CT]
            c_adab = c_all[:, 8:56]
            c_n1 = c_all[:, 56:64]
            c_n2 = c_all[:, 64:72]
            c_n3 = c_all[:, 72:80]
            c_cs4 = pp.tile([128, L], BF16, tag="c_cs4")
            c_ss4 = pp.tile([128, L], BF16, tag="c_ss4")
            xres = pp.tile([128, CT, LQ], FP32, tag="xres")
            ones_col = pp.tile([128, 1], BF16, tag="ones_col")
            ones_row = pp.tile([1, 128], BF16, tag="ones_row")
            eps_c = pp.tile([1, 1], FP32, tag="eps_c")
            nc.gpsimd.memset(ones_col, 1.0)
            nc.gpsimd.memset(ones_row, 1.0)
            nc.gpsimd.memset(eps_c, EPS)
            modsT = pp.tile([128, 48], FP32, tag="modsT")
            silu_bf = pp.tile([128, CT], BF16, tag="silu_bf")
            silu_f8 = pp.tile([128, CT], F8, tag="silu_f8")
            w1eff = pp.tile([128, CT], FP32, tag="w1eff")
            w3eff = pp.tile([128, CT], FP32, tag="w3eff")
            # attn output accumulators (fp8, DR-paired; reused by cross attn)
            att2 = [pp.tile([128, 2, LQ], F8, tag=f"att{j}", name=f"att{j}")
                    for j in range(KP)]
            # cross K (hg,j layout) / V (natural, 256-key windows)
            kcT8 = [pp.tile([128, 2, L2], F8, tag=f"kc{g}", name=f"kcT{g}")
                    for g in range(4)]
            vcb8 = [pp.tile([128, 2, H, D + 1], F8, tag=f"vc{w}",
                            name=f"vcb{w}") for w in range(NWC)]

            def sh_sa(k):
                return modsT[:, 0 + k:1 + k]

            def g_sa(k):
                return modsT[:, 16 + k:17 + k]

            def sh_ml(k):
                return modsT[:, 24 + k:25 + k]

            def g_ml(k):
                return modsT[:, 40 + k:41 + k]

            with tc.tile_pool(name="pkv", bufs=1) as pkv:
                # V in 256-key windows: [128, j, head, D+1] fp8
                vsb8 = [pkv.tile([128, 2, H, D + 1], F8, tag=f"v{w}",
                                 name=f"v{w}") for w in range(NW)]
                qT8 = [pkv.tile([128, 2, LQ], F8, tag=f"qT{g}", name=f"qT{g}")
                       for g in range(4)]
                # head3 shadow: base-64 matmul window [64:128) with head2's
                # rows zeroed (PE ops only allow base partition 0/32/64)
                qz8 = [pkv.tile([128, 2, LQ], F8, tag=f"qz{g}", name=f"qz{g}")
                       for g in range(4)]
                # adaLN-A weights borrow the kT8 slots (unused until k_proj)
                # allocated in fetch order: scale_sa pieces (4..7) first
                ADA_ORD = [4, 5, 6, 7, 0, 1, 2, 3]
                wadaA_t = {}
                for i in ADA_ORD:
                    wadaA_t[i] = pkv.tile([128, L], F8, tag="wadaAx", bufs=6,
                                          name=f"wadaA{i}")
                # SA-phase tiles (former pat pool, merged so k_proj can
                # overlap the SA stream)
                w_sa = pkv.tile([128, KP, 2, C], F8, tag="w_sa")
                w_kv = pkv.tile([128, 3, 2, 2 * C], F8, tag="w_kv")
                a_t = pkv.tile([128, 3, 2, L2], F8, tag="a_t")
                wadaB_t = {}

                with tc.tile_pool(name="pqw", bufs=1) as pqw:
                    w_q = pqw.tile([128, KP, 2, C], F8, tag="wmain", bufs=2,
                                   name="w_q")
                    xsa2 = [[pqw.tile([128, 2, 512], F8, tag=f"xsa{j}_{lc}",
                                      name=f"xsa{j}_{lc}")
                             for j in range(KP)] for lc in range(LCH)]
                    xc = {}

                    def x_fetch(lc, q=None):
                        xc[lc] = pqw.tile([128, CT, 512], BF16, tag="xinc",
                                          bufs=3, name=f"xin{lc}")
                        (q or nc.sync).dma_start(out=xc[lc], in_=x_bf[:, :].rearrange(
                            "(k p) l -> p k l", p=128)[:, :, lc * 512:(lc + 1) * 512])

                    # ---- DMA issue order (SP FIFO) ----
                    x_fetch(0)
                    dma(out=c_all, in_=cst[:, :])
                    for i in ADA_ORD:
                        dma(out=wadaA_t[i],
                            in_=wadaA[:, :, i * 256:(i + 1) * 256])
                    dma(out=c_cs4, in_=cs4[:, :])
                    dma(out=c_ss4, in_=ss4[:, :])
                    x_fetch(1)
                    dma(out=w_q, in_=wq2[:, :, :, :])
                    x_fetch(2)

                    mark("norm1")
                    # ---- silu(t_mod) on Act directly ----
                    nc.scalar.activation(out=silu_bf, in_=c_tmod, func=AF.Silu)
                    nc.vector.tensor_copy(silu_f8, silu_bf)

                    pbs = {}

                    def norm1_ssq(lc):
                        pssq = ps.tile([1, 512], FP32, tag="pC", bufs=2,
                                       name=f"pssq{lc}")
                        for k in range(CT):
                            xsq = pqw.tile([128, 512], BF16, tag="xsq", bufs=1,
                                           name=f"xsq{lc}_{k}")
                            if k % 2 == 0:
                                nc.vector.tensor_mul(xsq, xc[lc][:, k, :],
                                                     xc[lc][:, k, :])
                            else:
                                nc.scalar.activation(out=xsq,
                                                     in_=xc[lc][:, k, :],
                                                     func=AF.Square)
                            nc.tensor.matmul(pssq, ones_col, xsq,
                                             start=(k == 0), stop=(k == CT - 1))
                        rstd = pqw.tile([1, 512], FP32, tag="rstd", bufs=2,
                                        name=f"rstd{lc}")
                        nc.scalar.activation(out=rstd, in_=pssq, func=AF.Sqrt,
                                             bias=eps_c, scale=1.0 / C)
                        rstd_bf = pqw.tile([1, 512], BF16, tag="rstd_bf", bufs=2,
                                           name=f"rstdb{lc}")
                        with nc.allow_low_precision(reason="rstd bf16, matches prior fp32-recip+bf16-copy"):
                            nc.vector.reciprocal(rstd_bf, rstd)
                        pb = ps.tile([128, 512], FP32, tag="pA", bufs=2,
                                     name=f"pbn1{lc}")
                        nc.tensor.matmul(pb, ones_row, rstd_bf, start=True, stop=True)
                        pbsb = pqw.tile([128, 512], BF16, tag="pbsb", bufs=2,
                                        name=f"pbsb{lc}")
                        nc.scalar.copy(out=pbsb, in_=pb)
                        pbs[lc] = pbsb

                    def mod1(lc):
                        for k in range(CT):
                            dst = xsa2[lc][k // 2][:, k % 2, :]
                            eng = nc.vector
                            eng.scalar_tensor_tensor(
                                out=dst, in0=xc[lc][:, k, :],
                                scalar=w1eff[:, k:k + 1], in1=pbs[lc],
                                op0=mybir.AluOpType.mult,
                                op1=mybir.AluOpType.mult)
                            if k % 2 == 0:
                                nc.scalar.activation(
                                    out=dst, in_=dst, func=AF.Identity,
                                    bias=sh_sa(k))
                            else:
                                nc.gpsimd.tensor_scalar(
                                    out=dst, in0=dst, scalar1=sh_sa(k),
                                    scalar2=None, op0=mybir.AluOpType.add)

                    norm1_ssq(0)

                    mark("adaLN")
                    # ---- adaLN part A: scale_sa first (w1eff path), then shift ----
                    pmA = ps.tile([128, 16], FP32, tag="pC", bufs=2, name="pmA")
                    for j in list(range(8, 16)) + list(range(8)):
                        for k in range(CT):
                            nc.tensor.matmul(pmA[:, j:j + 1],
                                             wadaA_t[j // 2][:, k * 256 + (j % 2) * 128:
                                                             k * 256 + (j % 2) * 128 + 128],
                                             silu_f8[:, k:k + 1],
                                             start=(k == 0), stop=(k == CT - 1))
                        if j == 15:
                            nc.vector.tensor_add(modsT[:, 8:16], pmA[:, 8:16],
                                                 c_adab[:, 8:16])
                            nc.vector.tensor_scalar(out=w1eff, in0=modsT[:, 8:16],
                                                    scalar1=1.0, scalar2=None,
                                                    op0=mybir.AluOpType.add)
                            nc.vector.tensor_mul(w1eff, w1eff, c_n1)
                    nc.vector.tensor_add(modsT[:, 0:8], pmA[:, 0:8],
                                         c_adab[:, 0:8])

                    mark("mod1")
                    mod1(0)
                    norm1_ssq(1)
                    w_k = pqw.tile([128, KP, 2, C], F8, tag="wmain", bufs=2,
                                   name="w_k")
                    w_v = pqw.tile([128, KP, 2, C], F8, tag="wmain", bufs=2,
                                   name="w_v")
                    dma(out=w_k, in_=wk2[:, :, :, :])
                    x_fetch(3)
                    dma(out=w_v, in_=wv2[:, :, :, :])

                    def proj_dr(out_psum, w, m, xcols, nkp=KP):
                        for kp in range(nkp):
                            nc.tensor.matmul(out_psum,
                                             w[:, kp, :, m * 128:(m + 1) * 128],
                                             xcols(kp),
                                             start=(kp == 0), stop=(kp == nkp - 1),
                                             perf_mode=DR)

                    kT8 = [pkv.tile([128, 2, L], F8, tag="kTx", bufs=4,
                                    name=f"kT{g}") for g in range(4)]
                    rope_rr = [0]

                    def rope_unit2(dst, pk0, pk1, cols, act_ok=True):
                        kb0 = pkv.tile([128, 512], BF16, tag="ropet", bufs=8,
                                       name="kb0")
                        nc.vector.tensor_copy(kb0, pk0)
                        kb1 = pkv.tile([128, 512], BF16, tag="ropet", bufs=8,
                                       name="kb1")
                        if act_ok:
                            nc.scalar.copy(out=kb1, in_=pk1)
                        else:
                            nc.vector.tensor_copy(kb1, pk1)
                        ma = pkv.tile([128, 512], BF16, tag="ropet", bufs=8,
                                      name="ma")
                        nc.vector.tensor_mul(ma, kb0, c_cs4[:, cols])
                        mb = pkv.tile([128, 512], BF16, tag="ropet", bufs=8,
                                      name="mb")
                        nc.vector.tensor_mul(mb, kb1, c_ss4[:, cols])
                        mc = pkv.tile([128, 512], BF16, tag="ropet", bufs=8,
                                      name="mc")
                        nc.vector.tensor_mul(mc, kb0, c_ss4[:, cols])
                        md = pkv.tile([128, 512], BF16, tag="ropet", bufs=8,
                                      name="md")
                        nc.vector.tensor_mul(md, kb1, c_cs4[:, cols])
                        rope_rr[0] ^= 1
                        if rope_rr[0]:
                            nc.vector.tensor_sub(dst[:, 0, cols], ma, mb)
                            nc.gpsimd.tensor_add(dst[:, 1, cols], mc, md)
                        else:
                            nc.gpsimd.tensor_sub(dst[:, 0, cols], ma, mb)
                            nc.vector.tensor_add(dst[:, 1, cols], mc, md)

                    def rope_unit(dst, pq2, cols):
                        # dst[:,0,cols] = pq2[:,0]*cos - pq2[:,1]*sin
                        # dst[:,1,cols] = pq2[:,0]*sin + pq2[:,1]*cos
                        kb0 = pkv.tile([128, 512], BF16, tag="ropet", bufs=8,
                                       name="kb0")
                        nc.vector.tensor_copy(kb0, pq2[:, 0, :])
                        kb1 = pkv.tile([128, 512], BF16, tag="ropet", bufs=8,
                                       name="kb1")
                        nc.scalar.copy(out=kb1, in_=pq2[:, 1, :])
                        ma = pkv.tile([128, 512], BF16, tag="ropet", bufs=8,
                                      name="ma")
                        nc.vector.tensor_mul(ma, kb0, c_cs4[:, cols])
                        mb = pkv.tile([128, 512], BF16, tag="ropet", bufs=8,
                                      name="mb")
                        nc.vector.tensor_mul(mb, kb1, c_ss4[:, cols])
                        mc = pkv.tile([128, 512], BF16, tag="ropet", bufs=8,
                                      name="mc")
                        nc.vector.tensor_mul(mc, kb0, c_ss4[:, cols])
                        md = pkv.tile([128, 512], BF16, tag="ropet", bufs=8,
                                      name="md")
                        nc.vector.tensor_mul(md, kb1, c_cs4[:, cols])
                        # final adds alternate DVE / Pool to balance load
                        rope_rr[0] ^= 1
                        if rope_rr[0]:
                            nc.vector.tensor_sub(dst[:, 0, cols], ma, mb)
                            nc.gpsimd.tensor_add(dst[:, 1, cols], mc, md)
                        else:
                            nc.gpsimd.tensor_sub(dst[:, 0, cols], ma, mb)
                            nc.vector.tensor_add(dst[:, 1, cols], mc, md)

                    mark("q_proj")
                    # ====== q projection (own chunk = mod chunk 0) + rope ======
                    OWN = slice(0, LQ)
                    for hg in range(4):
                        pq2 = ps.tile([128, 2, LQ], FP32, tag="pQ", bufs=2,
                                      name=f"pq{hg}")
                        proj_dr(pq2[:, 0, :], w_q, hg * 2,
                                lambda kp: xsa2[0][kp][:, :, :])
                        proj_dr(pq2[:, 1, :], w_q, hg * 2 + 1,
                                lambda kp: xsa2[0][kp][:, :, :])
                        rope_unit(qT8[hg], pq2, OWN)
                        nc.vector.memset(qz8[hg][64:96, :, :], 0.0)
                        nc.scalar.copy(out=qz8[hg][96:128, :, :],
                                       in_=qT8[hg][96:128, :, :])
                        if hg == 0:
                            norm1_ssq(2)
                            mod1(1)
                        if hg == 1:
                            mod1(2)
                        if hg == 2:
                            norm1_ssq(3)
                        if hg == 3:
                            mod1(3)

                    mark("k_proj")
                    # ====== k projection + rope (hg0 now, hg1-3 in SA stream);
                    #        v units interleaved ======
                    dma(out=xres, in_=xq_f[:, :].rearrange(
                        "(k p) l -> p k l", p=128))
                    dma(out=w_kv, in_=wkv2[:, :, :, :])
                    dma(out=a_t, in_=aud2[:, :, :, :])
                    dma(out=w_sa, in_=wsa2[:, :, :, :])

                    def k_unit(hg, lc):
                        sl = slice(lc * 512, (lc + 1) * 512)
                        pka = ps.tile([128, 512], FP32, tag="pA", bufs=2,
                                      name=f"pk{hg}_{lc}a")
                        pkb = ps.tile([128, 512], FP32, tag="pA", bufs=2,
                                      name=f"pk{hg}_{lc}b")
                        proj_dr(pka, w_k, hg * 2,
                                lambda kp: xsa2[lc][kp][:, :, :])
                        proj_dr(pkb, w_k, hg * 2 + 1,
                                lambda kp: xsa2[lc][kp][:, :, :])
                        rope_unit2(kT8[hg], pka, pkb, sl,
                                   act_ok=(hg < 2))

                    def v_unit(t, g):
                        # t: 128-token chunk 0..15, g: channel half
                        w = t // 2
                        if g == 0 and t % 2 == 0:
                            nc.vector.memset(vsb8[w][:, :, :, D:D + 1], 1.0)
                        pv = ps.tile([128, 512], FP32, tag="pA", bufs=2,
                                     name=f"pv{t}_{g}")
                        for kp in range(KP):
                            nc.tensor.matmul(
                                pv, xsa2[t // 4][kp][:, :, (t % 4) * 128:
                                                     (t % 4) * 128 + 128],
                                w_v[:, kp, :, g * 512:(g + 1) * 512],
                                start=(kp == 0), stop=(kp == KP - 1),
                                perf_mode=DR)
                        dstv = vsb8[w][:, t % 2, g * 8:(g + 1) * 8, 0:D]
                        srcv = pv.rearrange("p (h d) -> p h d", h=8)
                        if t < 8:
                            nc.scalar.copy(out=dstv, in_=srcv)
                        else:
                            nc.vector.tensor_copy(dstv, srcv)

                    for lc in range(LCH):
                        k_unit(0, lc)
                        for t in range(lc * 2, lc * 2 + 2):
                            v_unit(t, 0)
                            v_unit(t, 1)
                    mark("v_proj")
                    for lc in range(LCH):
                        k_unit(1, lc)
                        for t in range(8 + lc * 2, 8 + lc * 2 + 2):
                            v_unit(t, 0)
                            v_unit(t, 1)

                    # ---- SA-phase helpers (cross kv, adaLN-B) ----
                    def adaB_fetch(i):
                        for hf in range(2):
                            wadaB_t[(i, hf)] = pkv.tile(
                                [128, CT // 2, 512], BF16, tag="wadaB",
                                bufs=2, name=f"wadaB{i}_{hf}")
                            dma(out=wadaB_t[(i, hf)],
                                in_=wadaB[i, :, hf * 4:(hf + 1) * 4, :])

                    def cross_kv_piece(i):
                        # i 0..7: kc tile (hg, j); i 8..15: vc (tchunk, ghalf)
                        if i < 8:
                            hg, j = divmod(i, 2)
                            pkc = ps.tile([128, L2], FP32, tag="pA", bufs=2,
                                          name=f"pkc{i}")
                            for kp in range(3):
                                nc.tensor.matmul(pkc,
                                                 w_kv[:, kp, :, i * 128:(i + 1) * 128],
                                                 a_t[:, kp, :, :],
                                                 start=(kp == 0), stop=(kp == 2),
                                                 perf_mode=DR)
                            nc.vector.tensor_copy(kcT8[hg][:, j, :], pkc)
                        else:
                            t, g = divmod(i - 8, 2)
                            if g == 0 and t % 2 == 0:
                                nc.vector.memset(
                                    vcb8[t // 2][:, :, :, D:D + 1], 1.0)
                            pvc = ps.tile([128, 512], FP32, tag="pA", bufs=2,
                                          name=f"pvc{i}")
                            for kp in range(3):
                                nc.tensor.matmul(
                                    pvc, a_t[:, kp, :, t * 128:(t + 1) * 128],
                                    w_kv[:, kp, :, C + g * 512:C + (g + 1) * 512],
                                    start=(kp == 0), stop=(kp == 2),
                                    perf_mode=DR)
                            nc.vector.tensor_copy(
                                vcb8[t // 2][:, t % 2, g * 8:(g + 1) * 8, 0:D],
                                pvc.rearrange("p (h d) -> p h d", h=8))

                    def adaB_piece(i):
                        j0 = 16 + 4 * i
                        pmB = ps.tile([128, 4], FP32, tag="pA", bufs=2,
                                      name=f"pmB{i}")
                        for jj in range(4):
                            for k in range(CT):
                                nc.tensor.matmul(pmB[:, jj:jj + 1],
                                                 wadaB_t[(i, k // 4)][:, k % 4,
                                                                      jj * 128:(jj + 1) * 128],
                                                 silu_bf[:, k:k + 1],
                                                 start=(k == 0), stop=(k == CT - 1))
                        nc.vector.tensor_add(modsT[:, j0:j0 + 4], pmB,
                                             c_adab[:, j0:j0 + 4])
                        if i == 5:
                            nc.vector.tensor_scalar(out=w3eff, in0=modsT[:, 32:40],
                                                    scalar1=1.0, scalar2=None,
                                                    op0=mybir.AluOpType.add)
                            nc.vector.tensor_mul(w3eff, w3eff, c_n3)

                    def proj_dr2(out_psum, w, m, xcols, nkp=KP):
                        for kp in range(nkp):
                            nc.tensor.matmul(out_psum,
                                             w[:, kp, :, m * 128:(m + 1) * 128],
                                             xcols(kp),
                                             start=(kp == 0),
                                             stop=(kp == nkp - 1),
                                             perf_mode=DR)

                    def sa_out_unit(m):
                        pso = ps.tile([128, LQ], FP32, tag="pA", bufs=2,
                                      name=f"pso{m}")
                        proj_dr2(pso, w_sa, m, lambda kp: att2[kp][:, :, :])
                        nc.vector.scalar_tensor_tensor(
                            out=xres[:, m, :], in0=pso, scalar=g_sa(m),
                            in1=xres[:, m, :],
                            op0=mybir.AluOpType.mult, op1=mybir.AluOpType.add)

                    mark("self_attn")
                    # ====== self-attention stream: item g = (h, w256) ======
                    LAG = 3
                    pos = {}
                    pexps = {}
                    pending = []  # (due_item, closure) in issue order

                    def sa_epilogue(h):
                        def run():
                            m = h // 2
                            rs = slice((h % 2) * 64, (h % 2) * 64 + 64)
                            po = pos.pop(h)
                            rec_bf = pp.tile([1, LQ], BF16, tag="rec_bf", bufs=2,
                                             name=f"recb{h}")
                            with nc.allow_low_precision(reason="softmax 1/sum bf16"):
                                nc.vector.reciprocal(rec_bf, po[64:65, :])
                            pbc = ps.tile([64, LQ], FP32, tag="pA", bufs=2,
                                          name=f"pbc{h}")
                            nc.tensor.matmul(pbc, ones_row[:, 0:64], rec_bf,
                                             start=True, stop=True)
                            rb_sb = pp.tile([64, LQ], BF16, tag="rb_sb", bufs=2,
                                            name=f"rb{h}")
                            nc.vector.tensor_copy(rb_sb, pbc)
                            nc.vector.tensor_mul(att2[m // 2][rs, m % 2, :],
                                                 po[0:64, :], rb_sb)
                        return run

                    def sa_po(h, w):
                        def run():
                            px = pexps.pop((h, w))
                            nc.tensor.matmul(pos[h], vsb8[w][:, :, h, :], px,
                                             start=(w == 0), stop=(w == NW - 1),
                                             perf_mode=DR)
                        return run

                    NIT = H * NW
                    for g in range(NIT + NW):
                        while pending and pending[0][0] <= g:
                            pending.pop(0)[1]()
                        if g >= NIT:
                            continue
                        h, w = divmod(g, NW)
                        hg = h // 4
                        rs = slice((h % 4) * 32, (h % 4) * 32 + 32)
                        if w == 0:
                            pos[h] = ps.tile([65, LQ], FP32, tag="pC", bufs=2,
                                             name=f"po{h}")
                            if h < 8:
                                adaB_fetch(h)
                        psc = ps.tile([128, 2, LQ], FP32, tag="pQ", bufs=2,
                                      name=f"psc{h}_{w}")
                        if h % 4 == 3:
                            rs = slice(64, 128)
                            qmov = qz8[hg]
                        else:
                            qmov = qT8[hg]
                        for jj in (0, 1):
                            t = 2 * w + jj
                            nc.tensor.matmul(psc[:, jj, :],
                                             kT8[hg][rs, :, t * 128:(t + 1) * 128],
                                             qmov[rs, :, :],
                                             start=True, stop=True,
                                             perf_mode=DR)
                        pexp = pkv.tile([128, 2, LQ], F8, tag="pexpS", bufs=7,
                                        name=f"pexp{h}_{w}")
                        nc.scalar.activation(out=pexp, in_=psc, func=AF.Exp,
                                             scale=KSC)
                        pexps[(h, w)] = pexp
                        pending.append((g + LAG, sa_po(h, w)))
                        # interleaved work: late k units, cross kv, adaLN-B
                        if w == 6 and h < 8:
                            k_unit(2 + h // 4, h % 4)
                        if w == NW - 1:
                            pending.append((g + LAG + 2, sa_epilogue(h)))
                            if 2 <= h < 10:
                                pending.append((g + LAG + 3, (lambda hh:
                                    lambda: cross_kv_piece(2 * (hh - 2)))(h)))
                                pending.append((g + LAG + 3, (lambda hh:
                                    lambda: cross_kv_piece(2 * (hh - 2) + 1))(h)))
                            if h >= 8:
                                pending.append((g + LAG + 4, (lambda hh:
                                    lambda: adaB_piece(hh - 8))(h)))
                    while pending:
                        pending.pop(0)[1]()

                mark("sa_out")
                # ====== sa_out (needs ALL heads' att2) + norm2 ssq ======
                for m in range(CT):
                    sa_out_unit(m)
                pssq_n2 = ps.tile([1, LQ], FP32, tag="pC", bufs=2, name="pssq_n2")
                for m in range(CT):
                    xsq = pp.tile([128, LQ], BF16, tag="rb_sb", bufs=2,
                                  name=f"xsqn2_{m}")
                    nc.scalar.activation(out=xsq, in_=xres[:, m, :],
                                         func=AF.Square)
                    nc.tensor.matmul(pssq_n2, ones_col, xsq,
                                     start=(m == 0), stop=(m == CT - 1))

            mark("cross")
            # ====== cross attention + MLP ======
            with tc.tile_pool(name="pca", bufs=1) as pca:
                w_qc = pca.tile([128, KP, 2, C], F8, tag="w_qc")
                dma(out=w_qc, in_=wqc2[:, :, :, :])
                w_ca = pca.tile([128, KP, 2, C], F8, tag="w_ca")
                dma(out=w_ca, in_=wca2[:, :, :, :])
                wgh_t, wgl_t, wuh_t, wul_t = {}, {}, {}, {}

                def gu_fetch(mg):
                    for dd, src_, nm in ((wgh_t, wgh, "gh"), (wgl_t, wgl, "gl"),
                                         (wuh_t, wuh, "uh"), (wul_t, wul, "ul")):
                        dd[mg] = pca.tile([128, KP, 2, 512], F8, tag="wgu", bufs=8,
                                          name=f"w{nm}{mg}")
                        dma(out=dd[mg], in_=src_[mg])

                gu_fetch(0)
                gu_fetch(1)

                # norm2 (no modulation) -> xnb2 fp8 DR-paired
                xnb2 = [pca.tile([128, 2, LQ], F8, tag=f"xn{j}", name=f"xnb{j}")
                        for j in range(KP)]
                rstd2 = pca.tile([1, LQ], FP32, tag="rstd", bufs=2,
                                 name="rstd_n2")
                nc.scalar.activation(out=rstd2, in_=pssq_n2, func=AF.Sqrt,
                                     bias=eps_c, scale=1.0 / C)
                rstd2_bf = pca.tile([1, LQ], BF16, tag="rstd_bf", bufs=2,
                                    name="rstdb_n2")
                with nc.allow_low_precision(reason="rstd bf16"):
                    nc.vector.reciprocal(rstd2_bf, rstd2)
                pb2 = ps.tile([128, LQ], FP32, tag="pC", bufs=2, name="pb_n2")
                nc.tensor.matmul(pb2, ones_row, rstd2_bf, start=True, stop=True)
                pb2sb = pca.tile([128, LQ], BF16, tag="pb2sb", bufs=1,
                                 name="pb2sb")
                nc.scalar.copy(out=pb2sb, in_=pb2)
                for k in range(CT):
                    eng = nc.vector
                    eng.scalar_tensor_tensor(
                        out=xnb2[k // 2][:, k % 2, :], in0=xres[:, k, :],
                        scalar=c_n2[:, k:k + 1], in1=pb2sb,
                        op0=mybir.AluOpType.mult, op1=mybir.AluOpType.mult)

                # cross q projection into (hg, j) fp8 layout
                qcT8 = [pca.tile([128, 2, LQ], F8, tag=f"qc{g}", name=f"qcT{g}")
                        for g in range(4)]
                qcz8 = [pca.tile([128, 2, LQ], F8, tag=f"qcz{g}", name=f"qcz{g}")
                        for g in range(4)]

                def qc_unit(hg, j):
                    pq = ps.tile([128, LQ], FP32, tag="pA", bufs=2,
                                 name=f"pqc{hg}_{j}")
                    proj_dr2(pq, w_qc, hg * 2 + j, lambda kp: xnb2[kp][:, :, :])
                    nc.scalar.copy(out=qcT8[hg][:, j, :], in_=pq)
                    if j == 0:
                        nc.vector.memset(qcz8[hg][64:96, :, :], 0.0)
                    nc.scalar.copy(out=qcz8[hg][96:128, j, :],
                                   in_=pq[96:128, :])

                for hg, j in ((0, 0), (0, 1), (1, 0), (1, 1)):
                    qc_unit(hg, j)

                def ca_out_unit(m):
                    pco = ps.tile([128, LQ], FP32, tag="pA", bufs=2,
                                  name=f"pcao{m}")
                    proj_dr2(pco, w_ca, m, lambda kp: att2[kp][:, :, :])
                    nc.vector.tensor_add(xres[:, m, :], xres[:, m, :], pco)

                mark("cross_attn")
                # ====== cross-attention stream: item g = (h, w256) ======
                CLAG = 2
                pos = {}
                pexps = {}
                pending = []

                def ca_epilogue(h):
                    def run():
                        m = h // 2
                        rs = slice((h % 2) * 64, (h % 2) * 64 + 64)
                        po = pos.pop(h)
                        rec_bf = pp.tile([1, LQ], BF16, tag="rec_bf", bufs=2,
                                         name=f"recbc{h}")
                        with nc.allow_low_precision(reason="softmax 1/sum bf16"):
                            nc.vector.reciprocal(rec_bf, po[64:65, :])
                        pbc = ps.tile([64, LQ], FP32, tag="pA", bufs=2,
                                      name=f"pbcc{h}")
                        nc.tensor.matmul(pbc, ones_row[:, 0:64], rec_bf,
                                         start=True, stop=True)
                        rb_sb = pp.tile([64, LQ], BF16, tag="rb_sb", bufs=2,
                                        name=f"rbc{h}")
                        nc.vector.tensor_copy(rb_sb, pbc)
                        nc.vector.tensor_mul(att2[m // 2][rs, m % 2, :],
                                             po[0:64, :], rb_sb)
                    return run

                def ca_po(h, w):
                    def run():
                        px = pexps.pop((h, w))
                        nc.tensor.matmul(pos[h], vcb8[w][:, :, h, :], px,
                                         start=(w == 0), stop=(w == NWC - 1),
                                         perf_mode=DR)
                    return run

                NIT = H * NWC
                for g in range(NIT + 4):
                    while pending and pending[0][0] <= g:
                        pending.pop(0)[1]()
                    if g >= NIT:
                        continue
                    h, w = divmod(g, NWC)
                    hg = h // 4
                    rs = slice((h % 4) * 32, (h % 4) * 32 + 32)
                    if w == 0:
                        pos[h] = ps.tile([65, LQ], FP32, tag="pC", bufs=2,
                                         name=f"poc{h}")
                    psc = ps.tile([128, 2, LQ], FP32, tag="pQ", bufs=2,
                                  name=f"pscc{h}_{w}")
                    if h % 4 == 3:
                        rs = slice(64, 128)
                        qmov = qcz8[hg]
                    else:
                        qmov = qcT8[hg]
                    for jj in (0, 1):
                        t = 2 * w + jj
                        nc.tensor.matmul(psc[:, jj, :],
                                         kcT8[hg][rs, :, t * 128:(t + 1) * 128],
                                         qmov[rs, :, :],
                                         start=True, stop=True,
                                         perf_mode=DR)
                    pexp = pca.tile([128, 2, LQ], F8, tag="pexpC", bufs=4,
                                    name=f"pexpc{h}_{w}")
                    nc.scalar.activation(out=pexp, in_=psc, func=AF.Exp,
                                         scale=KSC)
                    pexps[(h, w)] = pexp
                    pending.append((g + CLAG, ca_po(h, w)))
                    if w == 0 and h % 2 == 0 and h // 2 + 4 < 8:
                        hgn, jn = divmod(h // 2 + 4, 2)
                        qc_unit(hgn, jn)
                    if w == NWC - 1:
                        pending.append((g + CLAG + 1, ca_epilogue(h)))
                        if h % 2 == 0 and 2 + h // 2 < 8:
                            pending.append((g + CLAG + 1, (lambda mg:
                                lambda: gu_fetch(mg))(2 + h // 2)))

                while pending:
                    pending.pop(0)[1]()

                mark("ca_out")
                # ca_out (needs ALL heads' att2) + norm3 ssq
                for m in range(CT):
                    ca_out_unit(m)
                pssq3 = ps.tile([1, LQ], FP32, tag="pC", bufs=2, name="pssq_n3")
                for m in range(CT):
                    xsq = pca.tile([128, LQ], BF16, tag="xsq2", bufs=2,
                                   name=f"xsq3_{m}")
                    nc.scalar.activation(out=xsq, in_=xres[:, m, :],
                                         func=AF.Square)
                    nc.tensor.matmul(pssq3, ones_col, xsq,
                                     start=(m == 0), stop=(m == CT - 1))

                mark("mlp_norm")
                # norm3 + modulation -> bf16, then hi/lo fp8 split
                xmb = [pca.tile([128, LQ], BF16, tag=f"xm{k}", name=f"xmb{k}")
                       for k in range(CT)]
                xh2 = [pca.tile([128, 2, LQ], F8, tag=f"xh{j}", name=f"xh{j}")
                       for j in range(KP)]
                xl2 = [pca.tile([128, 2, LQ], F8, tag=f"xl{j}", name=f"xl{j}")
                       for j in range(KP)]
                x64 = [pca.tile([128, 2, LQ], F8, tag=f"x6{j}", name=f"x6{j}")
                       for j in range(KP)]
                rstd3 = pca.tile([1, LQ], FP32, tag="rstd", bufs=2,
                                 name="rstd_n3")
                nc.scalar.activation(out=rstd3, in_=pssq3, func=AF.Sqrt,
                                     bias=eps_c, scale=1.0 / C)
                rstd3_bf = pca.tile([1, LQ], BF16, tag="rstd_bf", bufs=2,
                                    name="rstdb_n3")
                with nc.allow_low_precision(reason="rstd bf16"):
                    nc.vector.reciprocal(rstd3_bf, rstd3)
                pb3 = ps.tile([128, LQ], FP32, tag="pC", bufs=2, name="pb_n3")
                nc.tensor.matmul(pb3, ones_row, rstd3_bf, start=True, stop=True)
                pb3sb = pca.tile([128, LQ], BF16, tag="pb2sb", bufs=1,
                                 name="pb3sb")
                nc.scalar.copy(out=pb3sb, in_=pb3)
                for k in range(CT):
                    nc.vector.scalar_tensor_tensor(
                        out=xmb[k], in0=xres[:, k, :], scalar=w3eff[:, k:k + 1],
                        in1=pb3sb,
                        op0=mybir.AluOpType.mult, op1=mybir.AluOpType.mult)
                    nc.gpsimd.tensor_scalar(out=xmb[k], in0=xmb[k],
                                            scalar1=sh_ml(k), scalar2=None,
                                            op0=mybir.AluOpType.add)
                    hi = xh2[k // 2][:, k % 2, :]
                    lo = xl2[k // 2][:, k % 2, :]
                    eh = nc.gpsimd if k % 2 == 0 else nc.vector
                    eh.tensor_copy(hi, xmb[k])
                    eh.tensor_sub(lo, xmb[k], hi)
                    nc.scalar.activation(out=x64[k // 2][:, k % 2, :],
                                         in_=xmb[k], func=AF.Identity,
                                         scale=1.0 / LOSC)

                mark("gate_up")
                # h2: fp8 DR-paired ffn activations
                h2 = [pca.tile([128, 2, LQ], F8, tag=f"h{t}", name=f"h2_{t}")
                      for t in range(FF // 256)]
                h64_2 = [pca.tile([128, 2, LQ], F8, tag=f"h6{t}", name=f"h64_{t}")
                         for t in range(FF // 256)]
                wdh_t, wdl_t = {}, {}

                def down_fetch(m):
                    wdh_t[m] = pca.tile([128, 16, 2, 128], F8, tag="wdw", bufs=4,
                                        name=f"wdh{m}")
                    dma(out=wdh_t[m], in_=wdh[m])
                    wdl_t[m] = pca.tile([128, 16, 2, 128], F8, tag="wdw", bufs=4,
                                        name=f"wdl{m}")
                    dma(out=wdl_t[m], in_=wdl[m])

                def dr_hilo(p1, wh, wl, mi, xlo=True):
                    # Xh*Wh + (X/64)*(Wl*64) [+ Xl*Wh], all at true scale
                    ms = slice(mi * 128, (mi + 1) * 128)
                    for kp in range(KP):
                        nc.tensor.matmul(p1, wh[:, kp, :, ms], xh2[kp][:, :, :],
                                         start=(kp == 0), stop=False, perf_mode=DR)
                    for kp in range(KP):
                        nc.tensor.matmul(p1, wl[:, kp, :, ms], x64[kp][:, :, :],
                                         start=False, stop=(not xlo and kp == KP - 1),
                                         perf_mode=DR)
                    if xlo:
                        for kp in range(KP):
                            nc.tensor.matmul(p1, wh[:, kp, :, ms], xl2[kp][:, :, :],
                                             start=False, stop=(kp == KP - 1),
                                             perf_mode=DR)

                for mg in range(8):
                    if mg >= 6:
                        down_fetch(mg - 6)
                    for mi in range(4):
                        pgu = ps.tile([128, 2 * LQ], FP32, tag="pQ", bufs=2,
                                      name=f"pgu{mg}_{mi}")
                        p1g = pgu[:, 0:LQ]
                        p1u = pgu[:, LQ:2 * LQ]
                        dr_hilo(p1g, wgh_t[mg], wgl_t[mg], mi)
                        sgl = pca.tile([128, LQ], BF16, tag="sgb", bufs=2,
                                       name=f"sgl{mg}_{mi}")
                        nc.scalar.activation(out=sgl, in_=p1g, func=AF.Silu)
                        dr_hilo(p1u, wuh_t[mg], wul_t[mg], mi)
                        t = mg * 4 + mi
                        nc.vector.tensor_mul(h2[t // 2][:, t % 2, :], sgl, p1u)
                        h64 = h64_2[t // 2][:, t % 2, :]
                        nc.scalar.activation(
                            out=h64, in_=h2[t // 2][:, t % 2, :],
                            func=AF.Identity, scale=1.0 / LOSC)

                mark("down")
                # down proj: P1 = H*Wdh + H64*Wdl64; out = P1*g + xres
                for m in range(CT):
                    if m + 2 < CT:
                        down_fetch(m + 2)
                    pd1 = ps.tile([128, LQ], FP32, tag="pA", bufs=2, name=f"pd1{m}")
                    for fp in range(16):
                        nc.tensor.matmul(pd1, wdh_t[m][:, fp, :, :],
                                         h2[fp][:, :, :],
                                         start=(fp == 0), stop=False,
                                         perf_mode=DR)
                    for fp in range(16):
                        nc.tensor.matmul(pd1, wdl_t[m][:, fp, :, :],
                                         h64_2[fp][:, :, :],
                                         start=False, stop=(fp == 15),
                                         perf_mode=DR)
                    of = pca.tile([128, LQ], FP32, tag="of", bufs=2, name=f"of{m}")
                    nc.vector.scalar_tensor_tensor(
                        out=of, in0=pd1, scalar=g_ml(m), in1=xres[:, m, :],
                        op0=mybir.AluOpType.mult, op1=mybir.AluOpType.add)
                    dma(out=outT[m * 128:(m + 1) * 128, :], in_=of)

    nc.compile()
    return nc


def _dr_perm(rope: bool):
    # column order for the (hg, j) DR layout: new col o=(hg*2+j)*128+p
    # takes original W column idx[o].
    idx = np.zeros(C, dtype=np.int64)
    for hg in range(4):
        for j in range(2):
            for p in range(128):
                h = hg * 4 + p // 32
                d = j * 32 + p % 32
                if rope:
                    # rotate-half pair i=(d%32): real=2i, imag=2i+1
                    c0 = h * 64 + 2 * (d % 32) + (0 if d < 32 else 1)
                else:
                    c0 = h * 64 + d
                idx[(hg * 2 + j) * 128 + p] = c0
    return idx


def _bf(a):
    return np.ascontiguousarray(a).astype(ml_dtypes.bfloat16)


def _f8(a):
    return np.ascontiguousarray(a).astype(F8NP)


def _dr_pack(W):
    # [n_in, n_out] -> [128, n_in//256, 2, n_out]
    n_in, n_out = W.shape
    kp = n_in // 256
    return W.reshape(kp, 2, 128, n_out).transpose(2, 0, 1, 3)


def _hilo(W):
    hi = W.astype(F8NP)
    lo = ((W - hi.astype(np.float32)) * LOSC).astype(F8NP)
    return hi, lo


def _prep_shared(W_qkv, W_sa_out, W_q, W_kv, W_ca_out, W_gate, W_up, W_down,
                 adaLN_W, adaLN_b, norm1_w, norm2_w, norm3_w):
    idx_r = _dr_perm(True)
    idx_n = _dr_perm(False)
    wq = W_qkv[:, 0:C][:, idx_r]
    wk = W_qkv[:, C:2 * C][:, idx_r]
    wv = W_qkv[:, 2 * C:3 * C]
    wqc = np.asarray(W_q, np.float32)[:, idx_n]
    wkv = np.concatenate([np.asarray(W_kv, np.float32)[:, 0:C][:, idx_n],
                          np.asarray(W_kv, np.float32)[:, C:2 * C]], axis=1)

    def pack8(W):
        return _f8(_dr_pack(np.asarray(W, np.float32)))

    wgh_, wgl_ = _hilo(np.asarray(W_gate, np.float32))
    wuh_, wul_ = _hilo(np.asarray(W_up, np.float32))
    wdh_, wdl_ = _hilo(np.asarray(W_down, np.float32))

    def mlp_pack(w8):  # fp8 [C, FF] -> [8 mg][128, kp, 2, 512]
        d = _dr_pack(w8.astype(np.float32)).astype(F8NP)
        return np.ascontiguousarray(d.reshape(128, KP, 2, 8, 512)
                                    .transpose(3, 0, 1, 2, 4))

    def down_pack(w8):  # fp8 [FF, C] -> [8 m][128, 16 fp, 2, 128]
        d = _dr_pack(w8.astype(np.float32)).astype(F8NP)
        return np.ascontiguousarray(d.reshape(128, 16, 2, CT, 128)
                                    .transpose(3, 0, 1, 2, 4))

    wada = np.asarray(adaLN_W, np.float32).reshape(CT, 128, 48, 128)
    wadaA_h = wada[:, :, 0:16, :].transpose(1, 0, 2, 3).reshape(128, CT, 2048)
    wadaB_h = np.stack([
        wada[:, :, 16 + 4 * i:20 + 4 * i, :].transpose(1, 0, 2, 3)
        .reshape(128, CT, 512) for i in range(8)])

    sh = {
        "wq2": pack8(wq), "wk2": pack8(wk), "wv2": pack8(wv),
        "wsa2": pack8(W_sa_out), "wqc2": pack8(wqc), "wkv2": pack8(wkv),
        "wca2": pack8(W_ca_out),
        "wgh": mlp_pack(wgh_), "wgl": mlp_pack(wgl_),
        "wuh": mlp_pack(wuh_), "wul": mlp_pack(wul_),
        "wdh": down_pack(wdh_), "wdl": down_pack(wdl_),
        "wadaA": _f8(wadaA_h), "wadaB": _bf(wadaB_h),
        "cst_base": np.concatenate([
            np.asarray(adaLN_b, np.float32).reshape(48, 128).T,
            np.asarray(norm1_w, np.float32).reshape(8, 128).T,
            np.asarray(norm2_w, np.float32).reshape(8, 128).T,
            np.asarray(norm3_w, np.float32).reshape(8, 128).T], axis=1),
    }
    return sh


def make_in_maps(x, t_mod, audio_context, freqs_cos, freqs_sin,
                 norm1_w, norm2_w, norm3_w,
                 W_qkv, W_sa_out, W_q, W_kv, W_ca_out,
                 W_gate, W_up, W_down, adaLN_W, adaLN_b):
    sh = _prep_shared(W_qkv, W_sa_out, W_q, W_kv, W_ca_out, W_gate, W_up,
                      W_down, adaLN_W, adaLN_b, norm1_w, norm2_w, norm3_w)
    cosT = np.ascontiguousarray(np.asarray(freqs_cos, np.float32).T)
    sinT = np.ascontiguousarray(np.asarray(freqs_sin, np.float32).T)

    in_maps = []
    for core in range(NCORE):
        b, j = divmod(core, 4)
        # roll the token axis so this core's own 512 tokens sit at [0, LQ)
        xT = np.roll(np.ascontiguousarray(np.asarray(x, np.float32)[b].T),
                     -j * LQ, axis=1)
        m = {k: v for k, v in sh.items() if k != "cst_base"}
        m["x_bf"] = _bf(xT)
        m["xq_f"] = np.ascontiguousarray(xT[:, 0:LQ])
        cr = np.roll(cosT, -j * LQ, axis=1)
        sr = np.roll(sinT, -j * LQ, axis=1)
        m["cs4"] = _bf(np.concatenate([cr, cr, cr, cr], axis=0))
        m["ss4"] = _bf(np.concatenate([sr, sr, sr, sr], axis=0))
        m["aud2"] = _f8(_dr_pack(
            np.ascontiguousarray(np.asarray(audio_context, np.float32)[b].T)))
        m["cst"] = np.ascontiguousarray(np.concatenate(
            [np.asarray(t_mod, np.float32)[b].reshape(8, 128).T,
             sh["cst_base"]], axis=1))
        in_maps.append(m)
    return in_maps


_NC_CACHE = None


def _get_nc():
    global _NC_CACHE
    if _NC_CACHE is None:
        _NC_CACHE = build_bass()
    return _NC_CACHE


def kernel(**inputs):
    nc = _get_nc()
    inputs = {k: np.asarray(v) for k, v in inputs.items()}
    in_maps = make_in_maps(**inputs)
    res = run_bass_kernel_spmd(nc, in_maps, list(range(NCORE)))
    out = np.zeros((B, L, C), np.float32)
    for core in range(NCORE):
        b, j = divmod(core, 4)
        out[b, j * LQ:(j + 1) * LQ, :] = res.results[core]["outT"].T
    return out


# revision 62
# speedup vs baseline: 1.0002x; 1.0002x over previous
"""Trainium2 Bass kernel for nn_ExpressionModel (dense DiT-style transformer block).

Sharding: 8 cores = 2 (batch) x 4 (sequence chunks of 512 tokens).
Each core computes the full block for its 512 query tokens; K/V projections
for the full 2048-token batch are duplicated across the 4 cores of a batch
(no collectives needed).

Key layout trick (vs the previous revision): q/k projection output tiles are
(head-group hg of 4 heads, j) with j in {0,1} the rotate-half block of the
head dim; partition p = (h%4)*32 + d%32. The rope "swap" partner of
partition p in block j is partition p in block 1-j of the SAME tile pair --
no swapped-weight second projection and no partition shuffles. The j dim
doubles as the fp8 DoubleRow pairing dim, so:
  - scores run fp8-DR [32,2]-stationary (0.5 cyc/col vs 1.0 bf16). PE
    operand base partitions must be 0/32/64, so the 4th head of each group
    reads a base-64 [64,2] window against a shadow q tile (qz8/qcz8) whose
    sibling-head rows are zeroed.
  - the exp writes probs as fp8 [128, 2, LQ] and p@V runs one fp8-DR
    matmul per 256-key window (vsb [128, 2, H, D+1]) -- 4x cheaper than
    bf16 128-key chunks.
Engine budget: Act's exps are the hard floor (~1038ns per [128,1024]; SA
133us + CA 33us); everything else is spread over DVE/Pool/Act by measured
cost (Pool cannot touch PSUM, scalar_tensor_tensor is DVE-only). DMA is a
single serial ~360GB/s resource, so the preamble queue order is arrival-
priority: x0, consts, adaLN-A (fp8), cos/sin, w_q, x1, x2, w_k, x3, w_v.
k head-groups 2-3 and the audio cross-K/V stream inside the self-attn
exp stream; adaLN-B columns stream weight-stationary the same way.
The MLP keeps the hi+lo fp8 split (T ~ T_hi + T_lo/64) on gate/up (3-pass)
and down (2-pass) -- measured HW rel err 0.0138 of the 0.02 budget; 2-pass
gate/up variants model out to ~0.019 on HW, too close to ship. Silu is a
single Act op (table includes copy/identity so no thrash with h64 scaling).
"""

import numpy as np
import ml_dtypes

import concourse.bass as bass
import concourse.tile as tile
from concourse import bacc, mybir
from concourse.bass_utils import run_bass_kernel_spmd

FP32 = mybir.dt.float32
BF16 = mybir.dt.bfloat16
F8 = mybir.dt.float8e4
DR = mybir.MatmulPerfMode.DoubleRow
F8NP = ml_dtypes.float8_e4m3

STAGE_MARKS = []  # (instruction-id watermark, stage name) — profiling aid

B, L, C = 2, 2048, 1024
H, D = 16, 64
L2, TD = 512, 768
FF = 4096
EPS = 1e-6
NCORE = 8
LQ = 512            # query tokens per core
CT = C // 128       # 8 C partition-tiles
KP = C // 256       # 4 DoubleRow contraction pairs over C
LCH = L // 512      # 4 512-token chunks
KSC = 1.0 / 8.0     # 1/sqrt(D)
LOSC = 64.0         # hi/lo split scale
NW = L // 256       # 8 256-key windows (self attn)
NWC = L2 // 256     # 2 windows (cross attn)

AF = mybir.ActivationFunctionType


def build_bass():
    nc = bacc.Bacc("TRN2", target_bir_lowering=False, debug=False)
    STAGE_MARKS.clear()

    def mark(stage):
        STAGE_MARKS.append((nc.next_id(), stage))

    def dma(out, in_):
        return nc.sync.dma_start(out=out, in_=in_)

    def din(name, shape, dt):
        return nc.dram_tensor(name, list(shape), dt, kind="ExternalInput")

    # --- inputs ---
    x_bf = din("x_bf", (C, L), BF16)            # x[b].T, bf16
    xq_f = din("xq_f", (C, LQ), FP32)           # own-chunk x[b].T, fp32 residual
    aud2 = din("aud2", (128, 3, 2, L2), F8)     # audio.T fp8 DR-paired
    cst = din("cst", (128, 80), FP32)           # tmod|adab|n1|n2|n3
    cs4 = din("cs4", (128, L), BF16)            # cos[p%32] rows
    ss4 = din("ss4", (128, L), BF16)            # sin[p%32] rows (plain)
    wadaA = din("wadaA", (128, CT, 2048), F8)      # adaLN W cols j0..15
    wadaB = din("wadaB", (8, 128, CT, 512), BF16)  # adaLN W cols j16..47
    wq2 = din("wq2", (128, KP, 2, C), F8)       # W_qkv q, (hg,j) DR layout
    wk2 = din("wk2", (128, KP, 2, C), F8)
    wv2 = din("wv2", (128, KP, 2, C), F8)       # v natural
    wsa2 = din("wsa2", (128, KP, 2, C), F8)
    wqc2 = din("wqc2", (128, KP, 2, C), F8)     # cross q, (hg,j) layout
    wkv2 = din("wkv2", (128, 3, 2, 2 * C), F8)  # K half (hg,j), V natural
    wca2 = din("wca2", (128, KP, 2, C), F8)
    wgh = din("wgh", (8, 128, KP, 2, 512), F8)  # MLP weights hi/lo fp8
    wgl = din("wgl", (8, 128, KP, 2, 512), F8)
    wuh = din("wuh", (8, 128, KP, 2, 512), F8)
    wul = din("wul", (8, 128, KP, 2, 512), F8)
    wdh = din("wdh", (CT, 128, 16, 2, 128), F8)
    wdl = din("wdl", (CT, 128, 16, 2, 128), F8)

    outT = nc.dram_tensor("outT", [C, LQ], FP32, kind="ExternalOutput")

    with tile.TileContext(nc) as tc:
        with (
            tc.tile_pool(name="pp", bufs=1) as pp,              # persistent
            tc.tile_pool(name="ps", bufs=1, space="PSUM") as ps,
        ):
            # ---- persistent constants (one packed tile) ----
            c_all = pp.tile([128, 80], FP32, tag="c_all")
            c_tmod = c_all[:, 0:CT]
            c_adab = c_all[:, 8:56]
            c_n1 = c_all[:, 56:64]
            c_n2 = c_all[:, 64:72]
            c_n3 = c_all[:, 72:80]
            c_cs4 = pp.tile([128, L], BF16, tag="c_cs4")
            c_ss4 = pp.tile([128, L], BF16, tag="c_ss4")
            xres = pp.tile([128, CT, LQ], FP32, tag="xres")
            ones_col = pp.tile([128, 1], BF16, tag="ones_col")
            ones_row = pp.tile([1, 128], BF16, tag="ones_row")
            eps_c = pp.tile([1, 1], FP32, tag="eps_c")
            nc.gpsimd.memset(ones_col, 1.0)
            nc.gpsimd.memset(ones_row, 1.0)
            nc.gpsimd.memset(eps_c, EPS)
            modsT = pp.tile([128, 48], FP32, tag="modsT")
            silu_bf = pp.tile([128, CT], BF16, tag="silu_bf")
            silu_f8 = pp.tile([128, CT], F8, tag="silu_f8")
            w1eff = pp.tile([128, CT], FP32, tag="w1eff")
            w3eff = pp.tile([128, CT], FP32, tag="w3eff")
            # attn output accumulators (fp8, DR-paired; reused by cross attn)
            att2 = [pp.tile([128, 2, LQ], F8, tag=f"att{j}", name=f"att{j}")
                    for j in range(KP)]
            # cross K (hg,j layout) / V (natural, 256-key windows)
            kcT8 = [pp.tile([128, 2, L2], F8, tag=f"kc{g}", name=f"kcT{g}")
                    for g in range(4)]
            vcb8 = [pp.tile([128, 2, H, D + 1], F8, tag=f"vc{w}",
                            name=f"vcb{w}") for w in range(NWC)]

            def sh_sa(k):
                return modsT[:, 0 + k:1 + k]

            def g_sa(k):
                return modsT[:, 16 + k:17 + k]

            def sh_ml(k):
                return modsT[:, 24 + k:25 + k]

            def g_ml(k):
                return modsT[:, 40 + k:41 + k]

            with tc.tile_pool(name="pkv", bufs=1) as pkv:
                # V in 256-key windows: [128, j, head, D+1] fp8
                vsb8 = [pkv.tile([128, 2, H, D + 1], F8, tag=f"v{w}",
                                 name=f"v{w}") for w in range(NW)]
                qT8 = [pkv.tile([128, 2, LQ], F8, tag=f"qT{g}", name=f"qT{g}")
                       for g in range(4)]
                # head3 shadow: base-64 matmul window [64:128) with head2's
                # rows zeroed (PE ops only allow base partition 0/32/64)
                qz8 = [pkv.tile([128, 2, LQ], F8, tag=f"qz{g}", name=f"qz{g}")
                       for g in range(4)]
                # adaLN-A weights borrow the kT8 slots (unused until k_proj)
                # allocated in fetch order: scale_sa pieces (4..7) first
                ADA_ORD = [4, 5, 6, 7, 0, 1, 2, 3]
                wadaA_t = {}
                for i in ADA_ORD:
                    wadaA_t[i] = pkv.tile([128, L], F8, tag="wadaAx", bufs=6,
                                          name=f"wadaA{i}")
                # SA-phase tiles (former pat pool, merged so k_proj can
                # overlap the SA stream)
                w_sa = pkv.tile([128, KP, 2, C], F8, tag="w_sa")
                w_kv = pkv.tile([128, 3, 2, 2 * C], F8, tag="w_kv")
                a_t = pkv.tile([128, 3, 2, L2], F8, tag="a_t")
                wadaB_t = {}

                with tc.tile_pool(name="pqw", bufs=1) as pqw:
                    w_q = pqw.tile([128, KP, 2, C], F8, tag="wmain", bufs=2,
                                   name="w_q")
                    xsa2 = [[pqw.tile([128, 2, 512], F8, tag=f"xsa{j}_{lc}",
                                      name=f"xsa{j}_{lc}")
                             for j in range(KP)] for lc in range(LCH)]
                    xc = {}

                    def x_fetch(lc, q=None):
                        halves = []
                        for hf in range(2):
                            t = pqw.tile([128, CT // 2, 512], BF16, tag="xinc",
                                         bufs=6, name=f"xin{lc}_{hf}")
                            (q or nc.sync).dma_start(
                                out=t, in_=x_bf[:, :].rearrange(
                                    "(k p) l -> p k l", p=128)
                                [:, hf * 4:(hf + 1) * 4,
                                 lc * 512:(lc + 1) * 512])
                            halves.append(t)
                        xc[lc] = halves

                    def xck(lc, k):
                        return xc[lc][k // 4][:, k % 4, :]

                    # ---- DMA issue order (SP FIFO) ----
                    x_fetch(0)
                    dma(out=c_all, in_=cst[:, :])
                    for i in ADA_ORD:
                        dma(out=wadaA_t[i],
                            in_=wadaA[:, :, i * 256:(i + 1) * 256])
                    dma(out=c_cs4, in_=cs4[:, :])
                    dma(out=c_ss4, in_=ss4[:, :])
                    x_fetch(1)
                    dma(out=w_q, in_=wq2[:, :, :, :])
                    x_fetch(2)

                    mark("norm1")
                    # ---- silu(t_mod) on Act directly ----
                    nc.scalar.activation(out=silu_bf, in_=c_tmod, func=AF.Silu)
                    nc.vector.tensor_copy(silu_f8, silu_bf)

                    pbs = {}

                    def norm1_ssq(lc):
                        pssq = ps.tile([1, 512], FP32, tag="pC", bufs=2,
                                       name=f"pssq{lc}")
                        for k in range(CT):
                            xsq = pqw.tile([128, 512], BF16, tag="xsq", bufs=1,
                                           name=f"xsq{lc}_{k}")
                            if k % 2 == 0:
                                nc.vector.tensor_mul(xsq, xck(lc, k),
                                                     xck(lc, k))
                            else:
                                nc.scalar.activation(out=xsq,
                                                     in_=xck(lc, k),
                                                     func=AF.Square)
                            nc.tensor.matmul(pssq, ones_col, xsq,
                                             start=(k == 0), stop=(k == CT - 1))
                        rstd = pqw.tile([1, 512], FP32, tag="rstd", bufs=2,
                                        name=f"rstd{lc}")
                        nc.scalar.activation(out=rstd, in_=pssq, func=AF.Sqrt,
                                             bias=eps_c, scale=1.0 / C)
                        rstd_bf = pqw.tile([1, 512], BF16, tag="rstd_bf", bufs=2,
                                           name=f"rstdb{lc}")
                        with nc.allow_low_precision(reason="rstd bf16, matches prior fp32-recip+bf16-copy"):
                            nc.vector.reciprocal(rstd_bf, rstd)
                        pb = ps.tile([128, 512], FP32, tag="pA", bufs=2,
                                     name=f"pbn1{lc}")
                        nc.tensor.matmul(pb, ones_row, rstd_bf, start=True, stop=True)
                        pbsb = pqw.tile([128, 512], BF16, tag="pbsb", bufs=2,
                                        name=f"pbsb{lc}")
                        nc.scalar.copy(out=pbsb, in_=pb)
                        pbs[lc] = pbsb

                    def mod1(lc):
                        for k in range(CT):
                            dst = xsa2[lc][k // 2][:, k % 2, :]
                            eng = nc.vector
                            eng.scalar_tensor_tensor(
                                out=dst, in0=xck(lc, k),
                                scalar=w1eff[:, k:k + 1], in1=pbs[lc],
                                op0=mybir.AluOpType.mult,
                                op1=mybir.AluOpType.mult)
                            if k % 2 == 0:
                                nc.scalar.activation(
                                    out=dst, in_=dst, func=AF.Identity,
                                    bias=sh_sa(k))
                            else:
                                nc.gpsimd.tensor_scalar(
                                    out=dst, in0=dst, scalar1=sh_sa(k),
                                    scalar2=None, op0=mybir.AluOpType.add)

                    norm1_ssq(0)

                    mark("adaLN")
                    # ---- adaLN part A: scale_sa first (w1eff path), then shift ----
                    pmA = ps.tile([128, 16], FP32, tag="pC", bufs=2, name="pmA")
                    for j in list(range(8, 16)) + list(range(8)):
                        for k in range(CT):
                            nc.tensor.matmul(pmA[:, j:j + 1],
                                             wadaA_t[j // 2][:, k * 256 + (j % 2) * 128:
                                                             k * 256 + (j % 2) * 128 + 128],
                                             silu_f8[:, k:k + 1],
                                             start=(k == 0), stop=(k == CT - 1))
                        if j == 15:
                            nc.vector.tensor_add(modsT[:, 8:16], pmA[:, 8:16],
                                                 c_adab[:, 8:16])
                            nc.vector.tensor_scalar(out=w1eff, in0=modsT[:, 8:16],
                                                    scalar1=1.0, scalar2=None,
                                                    op0=mybir.AluOpType.add)
                            nc.vector.tensor_mul(w1eff, w1eff, c_n1)
                    nc.vector.tensor_add(modsT[:, 0:8], pmA[:, 0:8],
                                         c_adab[:, 0:8])

                    mark("mod1")
                    mod1(0)
                    norm1_ssq(1)
                    w_k = pqw.tile([128, KP, 2, C], F8, tag="wmain", bufs=2,
                                   name="w_k")
                    w_v = pqw.tile([128, KP, 2, C], F8, tag="wmain", bufs=2,
                                   name="w_v")
                    dma(out=w_k, in_=wk2[:, :, :, :])
                    x_fetch(3)
                    dma(out=w_v, in_=wv2[:, :, :, :])

                    def proj_dr(out_psum, w, m, xcols, nkp=KP):
                        for kp in range(nkp):
                            nc.tensor.matmul(out_psum,
                                             w[:, kp, :, m * 128:(m + 1) * 128],
                                             xcols(kp),
                                             start=(kp == 0), stop=(kp == nkp - 1),
                                             perf_mode=DR)

                    kT8 = [pkv.tile([128, 2, L], F8, tag="kTx", bufs=4,
                                    name=f"kT{g}") for g in range(4)]
                    rope_rr = [0]

                    def rope_unit2(dst, pk0, pk1, cols, act_ok=True):
                        kb0 = pkv.tile([128, 512], BF16, tag="ropet", bufs=8,
                                       name="kb0")
                        nc.vector.tensor_copy(kb0, pk0)
                        kb1 = pkv.tile([128, 512], BF16, tag="ropet", bufs=8,
                                       name="kb1")
                        if act_ok:
                            nc.scalar.copy(out=kb1, in_=pk1)
                        else:
                            nc.vector.tensor_copy(kb1, pk1)
                        ma = pkv.tile([128, 512], BF16, tag="ropet", bufs=8,
                                      name="ma")
                        nc.vector.tensor_mul(ma, kb0, c_cs4[:, cols])
                        mb = pkv.tile([128, 512], BF16, tag="ropet", bufs=8,
                                      name="mb")
                        nc.vector.tensor_mul(mb, kb1, c_ss4[:, cols])
                        mc = pkv.tile([128, 512], BF16, tag="ropet", bufs=8,
                                      name="mc")
                        nc.vector.tensor_mul(mc, kb0, c_ss4[:, cols])
                        md = pkv.tile([128, 512], BF16, tag="ropet", bufs=8,
                                      name="md")
                        nc.vector.tensor_mul(md, kb1, c_cs4[:, cols])
                        rope_rr[0] ^= 1
                        if rope_rr[0]:
                            nc.vector.tensor_sub(dst[:, 0, cols], ma, mb)
                            nc.gpsimd.tensor_add(dst[:, 1, cols], mc, md)
                        else:
                            nc.gpsimd.tensor_sub(dst[:, 0, cols], ma, mb)
                            nc.vector.tensor_add(dst[:, 1, cols], mc, md)

                    def rope_unit(dst, pq2, cols):
                        # dst[:,0,cols] = pq2[:,0]*cos - pq2[:,1]*sin
                        # dst[:,1,cols] = pq2[:,0]*sin + pq2[:,1]*cos
                        kb0 = pkv.tile([128, 512], BF16, tag="ropet", bufs=8,
                                       name="kb0")
                        nc.vector.tensor_copy(kb0, pq2[:, 0, :])
                        kb1 = pkv.tile([128, 512], BF16, tag="ropet", bufs=8,
                                       name="kb1")
                        nc.scalar.copy(out=kb1, in_=pq2[:, 1, :])
                        ma = pkv.tile([128, 512], BF16, tag="ropet", bufs=8,
                                      name="ma")
                        nc.vector.tensor_mul(ma, kb0, c_cs4[:, cols])
                        mb = pkv.tile([128, 512], BF16, tag="ropet", bufs=8,
                                      name="mb")
                        nc.vector.tensor_mul(mb, kb1, c_ss4[:, cols])
                        mc = pkv.tile([128, 512], BF16, tag="ropet", bufs=8,
                                      name="mc")
                        nc.vector.tensor_mul(mc, kb0, c_ss4[:, cols])
                        md = pkv.tile([128, 512], BF16, tag="ropet", bufs=8,
                                      name="md")
                        nc.vector.tensor_mul(md, kb1, c_cs4[:, cols])
                        # final adds alternate DVE / Pool to balance load
                        rope_rr[0] ^= 1
                        if rope_rr[0]:
                            nc.vector.tensor_sub(dst[:, 0, cols], ma, mb)
                            nc.gpsimd.tensor_add(dst[:, 1, cols], mc, md)
                        else:
                            nc.gpsimd.tensor_sub(dst[:, 0, cols], ma, mb)
                            nc.vector.tensor_add(dst[:, 1, cols], mc, md)

                    mark("q_proj")
                    # ====== q projection (own chunk = mod chunk 0) + rope ======
                    OWN = slice(0, LQ)
                    for hg in range(4):
                        pq2 = ps.tile([128, 2, LQ], FP32, tag="pQ", bufs=2,
                                      name=f"pq{hg}")
                        proj_dr(pq2[:, 0, :], w_q, hg * 2,
                                lambda kp: xsa2[0][kp][:, :, :])
                        proj_dr(pq2[:, 1, :], w_q, hg * 2 + 1,
                                lambda kp: xsa2[0][kp][:, :, :])
                        rope_unit(qT8[hg], pq2, OWN)
                        nc.vector.memset(qz8[hg][64:96, :, :], 0.0)
                        nc.scalar.copy(out=qz8[hg][96:128, :, :],
                                       in_=qT8[hg][96:128, :, :])
                        if hg == 0:
                            norm1_ssq(2)
                            mod1(1)
                        if hg == 1:
                            mod1(2)
                        if hg == 2:
                            norm1_ssq(3)
                        if hg == 3:
                            mod1(3)

                    mark("k_proj")
                    # ====== k projection + rope (hg0 now, hg1-3 in SA stream);
                    #        v units interleaved ======
                    dma(out=xres, in_=xq_f[:, :].rearrange(
                        "(k p) l -> p k l", p=128))
                    dma(out=w_kv, in_=wkv2[:, :, :, :])
                    dma(out=a_t, in_=aud2[:, :, :, :])
                    dma(out=w_sa, in_=wsa2[:, :, :, :])

                    def k_unit(hg, lc):
                        sl = slice(lc * 512, (lc + 1) * 512)
                        pka = ps.tile([128, 512], FP32, tag="pA", bufs=2,
                                      name=f"pk{hg}_{lc}a")
                        pkb = ps.tile([128, 512], FP32, tag="pA", bufs=2,
                                      name=f"pk{hg}_{lc}b")
                        proj_dr(pka, w_k, hg * 2,
                                lambda kp: xsa2[lc][kp][:, :, :])
                        proj_dr(pkb, w_k, hg * 2 + 1,
                                lambda kp: xsa2[lc][kp][:, :, :])
                        rope_unit2(kT8[hg], pka, pkb, sl,
                                   act_ok=(hg < 2))

                    def v_unit(t, g):
                        # t: 128-token chunk 0..15, g: channel half
                        w = t // 2
                        if g == 0 and t % 2 == 0:
                            nc.vector.memset(vsb8[w][:, :, :, D:D + 1], 1.0)
                        pv = ps.tile([128, 512], FP32, tag="pA", bufs=2,
                                     name=f"pv{t}_{g}")
                        for kp in range(KP):
                            nc.tensor.matmul(
                                pv, xsa2[t // 4][kp][:, :, (t % 4) * 128:
                                                     (t % 4) * 128 + 128],
                                w_v[:, kp, :, g * 512:(g + 1) * 512],
                                start=(kp == 0), stop=(kp == KP - 1),
                                perf_mode=DR)
                        dstv = vsb8[w][:, t % 2, g * 8:(g + 1) * 8, 0:D]
                        srcv = pv.rearrange("p (h d) -> p h d", h=8)
                        if t < 8:
                            nc.scalar.copy(out=dstv, in_=srcv)
                        else:
                            nc.vector.tensor_copy(dstv, srcv)

                    for lc in range(LCH):
                        k_unit(0, lc)
                        for t in range(lc * 2, lc * 2 + 2):
                            v_unit(t, 0)
                            v_unit(t, 1)
                    mark("v_proj")
                    for lc in range(LCH):
                        k_unit(1, lc)
                        for t in range(8 + lc * 2, 8 + lc * 2 + 2):
                            v_unit(t, 0)
                            v_unit(t, 1)

                    # ---- SA-phase helpers (cross kv, adaLN-B) ----
                    def adaB_fetch(i):
                        for hf in range(2):
                            wadaB_t[(i, hf)] = pkv.tile(
                                [128, CT // 2, 512], BF16, tag="wadaB",
                                bufs=2, name=f"wadaB{i}_{hf}")
                            dma(out=wadaB_t[(i, hf)],
                                in_=wadaB[i, :, hf * 4:(hf + 1) * 4, :])

                    def cross_kv_piece(i):
                        # i 0..7: kc tile (hg, j); i 8..15: vc (tchunk, ghalf)
                        if i < 8:
                            hg, j = divmod(i, 2)
                            pkc = ps.tile([128, L2], FP32, tag="pA", bufs=2,
                                          name=f"pkc{i}")
                            for kp in range(3):
                                nc.tensor.matmul(pkc,
                                                 w_kv[:, kp, :, i * 128:(i + 1) * 128],
                                                 a_t[:, kp, :, :],
                                                 start=(kp == 0), stop=(kp == 2),
                                                 perf_mode=DR)
                            nc.vector.tensor_copy(kcT8[hg][:, j, :], pkc)
                        else:
                            t, g = divmod(i - 8, 2)
                            if g == 0 and t % 2 == 0:
                                nc.vector.memset(
                                    vcb8[t // 2][:, :, :, D:D + 1], 1.0)
                            pvc = ps.tile([128, 512], FP32, tag="pA", bufs=2,
                                          name=f"pvc{i}")
                            for kp in range(3):
                                nc.tensor.matmul(
                                    pvc, a_t[:, kp, :, t * 128:(t + 1) * 128],
                                    w_kv[:, kp, :, C + g * 512:C + (g + 1) * 512],
                                    start=(kp == 0), stop=(kp == 2),
                                    perf_mode=DR)
                            nc.vector.tensor_copy(
                                vcb8[t // 2][:, t % 2, g * 8:(g + 1) * 8, 0:D],
                                pvc.rearrange("p (h d) -> p h d", h=8))

                    def adaB_piece(i):
                        j0 = 16 + 4 * i
                        pmB = ps.tile([128, 4], FP32, tag="pA", bufs=2,
                                      name=f"pmB{i}")
                        for jj in range(4):
                            for k in range(CT):
                                nc.tensor.matmul(pmB[:, jj:jj + 1],
                                                 wadaB_t[(i, k // 4)][:, k % 4,
                                                                      jj * 128:(jj + 1) * 128],
                                                 silu_bf[:, k:k + 1],
                                                 start=(k == 0), stop=(k == CT - 1))
                        nc.vector.tensor_add(modsT[:, j0:j0 + 4], pmB,
                                             c_adab[:, j0:j0 + 4])
                        if i == 5:
                            nc.vector.tensor_scalar(out=w3eff, in0=modsT[:, 32:40],
                                                    scalar1=1.0, scalar2=None,
                                                    op0=mybir.AluOpType.add)
                            nc.vector.tensor_mul(w3eff, w3eff, c_n3)

                    def proj_dr2(out_psum, w, m, xcols, nkp=KP):
                        for kp in range(nkp):
                            nc.tensor.matmul(out_psum,
                                             w[:, kp, :, m * 128:(m + 1) * 128],
                                             xcols(kp),
                                             start=(kp == 0),
                                             stop=(kp == nkp - 1),
                                             perf_mode=DR)

                    def sa_out_unit(m):
                        pso = ps.tile([128, LQ], FP32, tag="pA", bufs=2,
                                      name=f"pso{m}")
                        proj_dr2(pso, w_sa, m, lambda kp: att2[kp][:, :, :])
                        nc.vector.scalar_tensor_tensor(
                            out=xres[:, m, :], in0=pso, scalar=g_sa(m),
                            in1=xres[:, m, :],
                            op0=mybir.AluOpType.mult, op1=mybir.AluOpType.add)

                    mark("self_attn")
                    # ====== self-attention stream: item g = (h, w256) ======
                    LAG = 3
                    pos = {}
                    pexps = {}
                    pending = []  # (due_item, closure) in issue order

                    def sa_epilogue(h):
                        def run():
                            m = h // 2
                            rs = slice((h % 2) * 64, (h % 2) * 64 + 64)
                            po = pos.pop(h)
                            rec_bf = pp.tile([1, LQ], BF16, tag="rec_bf", bufs=2,
                                             name=f"recb{h}")
                            with nc.allow_low_precision(reason="softmax 1/sum bf16"):
                                nc.vector.reciprocal(rec_bf, po[64:65, :])
                            pbc = ps.tile([64, LQ], FP32, tag="pA", bufs=2,
                                          name=f"pbc{h}")
                            nc.tensor.matmul(pbc, ones_row[:, 0:64], rec_bf,
                                             start=True, stop=True)
                            rb_sb = pp.tile([64, LQ], BF16, tag="rb_sb", bufs=2,
                                            name=f"rb{h}")
                            nc.vector.tensor_copy(rb_sb, pbc)
                            nc.vector.tensor_mul(att2[m // 2][rs, m % 2, :],
                                                 po[0:64, :], rb_sb)
                        return run

                    def sa_po(h, w):
                        def run():
                            px = pexps.pop((h, w))
                            nc.tensor.matmul(pos[h], vsb8[w][:, :, h, :], px,
                                             start=(w == 0), stop=(w == NW - 1),
                                             perf_mode=DR)
                        return run

                    NIT = H * NW
                    for g in range(NIT + NW):
                        while pending and pending[0][0] <= g:
                            pending.pop(0)[1]()
                        if g >= NIT:
                            continue
                        h, w = divmod(g, NW)
                        hg = h // 4
                        rs = slice((h % 4) * 32, (h % 4) * 32 + 32)
                        if w == 0:
                            pos[h] = ps.tile([65, LQ], FP32, tag="pC", bufs=2,
                                             name=f"po{h}")
                            if h < 8:
                                adaB_fetch(h)
                        psc = ps.tile([128, 2, LQ], FP32, tag="pQ", bufs=2,
                                      name=f"psc{h}_{w}")
                        if h % 4 == 3:
                            rs = slice(64, 128)
                            qmov = qz8[hg]
                        else:
                            qmov = qT8[hg]
                        for jj in (0, 1):
                            t = 2 * w + jj
                            nc.tensor.matmul(psc[:, jj, :],
                                             kT8[hg][rs, :, t * 128:(t + 1) * 128],
                                             qmov[rs, :, :],
                                             start=True, stop=True,
                                             perf_mode=DR)
                        pexp = pkv.tile([128, 2, LQ], F8, tag="pexpS", bufs=7,
                                        name=f"pexp{h}_{w}")
                        nc.scalar.activation(out=pexp, in_=psc, func=AF.Exp,
                                             scale=KSC)
                        pexps[(h, w)] = pexp
                        pending.append((g + LAG, sa_po(h, w)))
                        # interleaved work: late k units, cross kv, adaLN-B
                        if w == 6 and h < 8:
                            k_unit(2 + h // 4, h % 4)
                        if w == NW - 1:
                            pending.append((g + LAG + 2, sa_epilogue(h)))
                            if 2 <= h < 10:
                                pending.append((g + LAG + 3, (lambda hh:
                                    lambda: cross_kv_piece(2 * (hh - 2)))(h)))
                                pending.append((g + LAG + 3, (lambda hh:
                                    lambda: cross_kv_piece(2 * (hh - 2) + 1))(h)))
                            if h >= 8:
                                pending.append((g + LAG + 4, (lambda hh:
                                    lambda: adaB_piece(hh - 8))(h)))
                    while pending:
                        pending.pop(0)[1]()

                mark("sa_out")
                # ====== sa_out (needs ALL heads' att2) + norm2 ssq (lagged) ======
                pssq_n2 = ps.tile([1, LQ], FP32, tag="pC", bufs=2, name="pssq_n2")

                def n2_ssq(m):
                    xsq = pp.tile([128, LQ], BF16, tag="rb_sb", bufs=2,
                                  name=f"xsqn2_{m}")
                    nc.scalar.activation(out=xsq, in_=xres[:, m, :],
                                         func=AF.Square)
                    nc.tensor.matmul(pssq_n2, ones_col, xsq,
                                     start=(m == 0), stop=(m == CT - 1))

                for m in range(CT):
                    sa_out_unit(m)
                for m in range(CT):
                    n2_ssq(m)

            mark("cross")
            # ====== cross attention + MLP ======
            with tc.tile_pool(name="pca", bufs=1) as pca:
                w_qc = pca.tile([128, KP, 2, C], F8, tag="w_qc")
                dma(out=w_qc, in_=wqc2[:, :, :, :])
                w_ca = pca.tile([128, KP, 2, C], F8, tag="w_ca")
                dma(out=w_ca, in_=wca2[:, :, :, :])
                wgh_t, wgl_t, wuh_t, wul_t = {}, {}, {}, {}

                def gu_fetch(mg):
                    for dd, src_, nm in ((wgh_t, wgh, "gh"), (wgl_t, wgl, "gl"),
                                         (wuh_t, wuh, "uh"), (wul_t, wul, "ul")):
                        dd[mg] = pca.tile([128, KP, 2, 512], F8, tag="wgu", bufs=8,
                                          name=f"w{nm}{mg}")
                        dma(out=dd[mg], in_=src_[mg])

                gu_fetch(0)
                gu_fetch(1)

                # norm2 (no modulation) -> xnb2 fp8 DR-paired
                xnb2 = [pca.tile([128, 2, LQ], F8, tag=f"xn{j}", name=f"xnb{j}")
                        for j in range(KP)]
                rstd2 = pca.tile([1, LQ], FP32, tag="rstd", bufs=2,
                                 name="rstd_n2")
                nc.scalar.activation(out=rstd2, in_=pssq_n2, func=AF.Sqrt,
                                     bias=eps_c, scale=1.0 / C)
                rstd2_bf = pca.tile([1, LQ], BF16, tag="rstd_bf", bufs=2,
                                    name="rstdb_n2")
                with nc.allow_low_precision(reason="rstd bf16"):
                    nc.vector.reciprocal(rstd2_bf, rstd2)
                pb2 = ps.tile([128, LQ], FP32, tag="pC", bufs=2, name="pb_n2")
                nc.tensor.matmul(pb2, ones_row, rstd2_bf, start=True, stop=True)
                pb2sb = pca.tile([128, LQ], BF16, tag="pb2sb", bufs=1,
                                 name="pb2sb")
                nc.scalar.copy(out=pb2sb, in_=pb2)
                for k in range(CT):
                    eng = nc.vector
                    eng.scalar_tensor_tensor(
                        out=xnb2[k // 2][:, k % 2, :], in0=xres[:, k, :],
                        scalar=c_n2[:, k:k + 1], in1=pb2sb,
                        op0=mybir.AluOpType.mult, op1=mybir.AluOpType.mult)

                # cross q projection into (hg, j) fp8 layout
                qcT8 = [pca.tile([128, 2, LQ], F8, tag=f"qc{g}", name=f"qcT{g}")
                        for g in range(4)]
                qcz8 = [pca.tile([128, 2, LQ], F8, tag=f"qcz{g}", name=f"qcz{g}")
                        for g in range(4)]

                def qc_unit(hg, j):
                    pq = ps.tile([128, LQ], FP32, tag="pA", bufs=2,
                                 name=f"pqc{hg}_{j}")
                    proj_dr2(pq, w_qc, hg * 2 + j, lambda kp: xnb2[kp][:, :, :])
                    nc.scalar.copy(out=qcT8[hg][:, j, :], in_=pq)
                    if j == 0:
                        nc.vector.memset(qcz8[hg][64:96, :, :], 0.0)
                    nc.scalar.copy(out=qcz8[hg][96:128, j, :],
                                   in_=pq[96:128, :])

                for hg, j in ((0, 0), (0, 1), (1, 0), (1, 1)):
                    qc_unit(hg, j)

                def ca_out_unit(m):
                    pco = ps.tile([128, LQ], FP32, tag="pA", bufs=2,
                                  name=f"pcao{m}")
                    proj_dr2(pco, w_ca, m, lambda kp: att2[kp][:, :, :])
                    nc.vector.tensor_add(xres[:, m, :], xres[:, m, :], pco)

                mark("cross_attn")
                # ====== cross-attention stream: item g = (h, w256) ======
                CLAG = 2
                pos = {}
                pexps = {}
                pending = []

                def ca_epilogue(h):
                    def run():
                        m = h // 2
                        rs = slice((h % 2) * 64, (h % 2) * 64 + 64)
                        po = pos.pop(h)
                        rec_bf = pp.tile([1, LQ], BF16, tag="rec_bf", bufs=2,
                                         name=f"recbc{h}")
                        with nc.allow_low_precision(reason="softmax 1/sum bf16"):
                            nc.vector.reciprocal(rec_bf, po[64:65, :])
                        pbc = ps.tile([64, LQ], FP32, tag="pA", bufs=2,
                                      name=f"pbcc{h}")
                        nc.tensor.matmul(pbc, ones_row[:, 0:64], rec_bf,
                                         start=True, stop=True)
                        rb_sb = pp.tile([64, LQ], BF16, tag="rb_sb", bufs=2,
                                        name=f"rbc{h}")
                        nc.vector.tensor_copy(rb_sb, pbc)
                        nc.vector.tensor_mul(att2[m // 2][rs, m % 2, :],
                                             po[0:64, :], rb_sb)
                    return run

                def ca_po(h, w):
                    def run():
                        px = pexps.pop((h, w))
                        nc.tensor.matmul(pos[h], vcb8[w][:, :, h, :], px,
                                         start=(w == 0), stop=(w == NWC - 1),
                                         perf_mode=DR)
                    return run

                NIT = H * NWC
                for g in range(NIT + 4):
                    while pending and pending[0][0] <= g:
                        pending.pop(0)[1]()
                    if g >= NIT:
                        continue
                    h, w = divmod(g, NWC)
                    hg = h // 4
                    rs = slice((h % 4) * 32, (h % 4) * 32 + 32)
                    if w == 0:
                        pos[h] = ps.tile([65, LQ], FP32, tag="pC", bufs=2,
                                         name=f"poc{h}")
                    psc = ps.tile([128, 2, LQ], FP32, tag="pQ", bufs=2,
                                  name=f"pscc{h}_{w}")
                    if h % 4 == 3:
                        rs = slice(64, 128)
                        qmov = qcz8[hg]
                    else:
                        qmov = qcT8[hg]
                    for jj in (0, 1):
                        t = 2 * w + jj
                        nc.tensor.matmul(psc[:, jj, :],
                                         kcT8[hg][rs, :, t * 128:(t + 1) * 128],
                                         qmov[rs, :, :],
                                         start=True, stop=True,
                                         perf_mode=DR)
                    pexp = pca.tile([128, 2, LQ], F8, tag="pexpC", bufs=4,
                                    name=f"pexpc{h}_{w}")
                    nc.scalar.activation(out=pexp, in_=psc, func=AF.Exp,
                                         scale=KSC)
                    pexps[(h, w)] = pexp
                    pending.append((g + CLAG, ca_po(h, w)))
                    if w == 0 and h % 2 == 0 and h // 2 + 4 < 8:
                        hgn, jn = divmod(h // 2 + 4, 2)
                        qc_unit(hgn, jn)
                    if w == NWC - 1:
                        pending.append((g + CLAG + 1, ca_epilogue(h)))
                        if h % 2 == 0 and 2 + h // 2 < 8:
                            pending.append((g + CLAG + 1, (lambda mg:
                                lambda: gu_fetch(mg))(2 + h // 2)))

                while pending:
                    pending.pop(0)[1]()

                mark("ca_out")
                # ca_out (needs ALL heads' att2) + norm3 ssq (lagged)
                pssq3 = ps.tile([1, LQ], FP32, tag="pC", bufs=2, name="pssq_n3")

                def n3_ssq(m):
                    xsq = pca.tile([128, LQ], BF16, tag="xsq2", bufs=2,
                                   name=f"xsq3_{m}")
                    nc.scalar.activation(out=xsq, in_=xres[:, m, :],
                                         func=AF.Square)
                    nc.tensor.matmul(pssq3, ones_col, xsq,
                                     start=(m == 0), stop=(m == CT - 1))

                for m in range(CT):
                    ca_out_unit(m)
                for m in range(CT):
                    n3_ssq(m)

                mark("mlp_norm")
                # norm3 + modulation -> bf16, then hi/lo fp8 split
                xmb = [pca.tile([128, LQ], BF16, tag=f"xm{k}", name=f"xmb{k}")
                       for k in range(CT)]
                xh2 = [pca.tile([128, 2, LQ], F8, tag=f"xh{j}", name=f"xh{j}")
                       for j in range(KP)]
                xl2 = [pca.tile([128, 2, LQ], F8, tag=f"xl{j}", name=f"xl{j}")
                       for j in range(KP)]
                x64 = [pca.tile([128, 2, LQ], F8, tag=f"x6{j}", name=f"x6{j}")
                       for j in range(KP)]
                rstd3 = pca.tile([1, LQ], FP32, tag="rstd", bufs=2,
                                 name="rstd_n3")
                nc.scalar.activation(out=rstd3, in_=pssq3, func=AF.Sqrt,
                                     bias=eps_c, scale=1.0 / C)
                rstd3_bf = pca.tile([1, LQ], BF16, tag="rstd_bf", bufs=2,
                                    name="rstdb_n3")
                with nc.allow_low_precision(reason="rstd bf16"):
                    nc.vector.reciprocal(rstd3_bf, rstd3)
                pb3 = ps.tile([128, LQ], FP32, tag="pC", bufs=2, name="pb_n3")
                nc.tensor.matmul(pb3, ones_row, rstd3_bf, start=True, stop=True)
                pb3sb = pca.tile([128, LQ], BF16, tag="pb2sb", bufs=1,
                                 name="pb3sb")
                nc.scalar.copy(out=pb3sb, in_=pb3)
                for k in range(CT):
                    nc.vector.scalar_tensor_tensor(
                        out=xmb[k], in0=xres[:, k, :], scalar=w3eff[:, k:k + 1],
                        in1=pb3sb,
                        op0=mybir.AluOpType.mult, op1=mybir.AluOpType.mult)
                    nc.gpsimd.tensor_scalar(out=xmb[k], in0=xmb[k],
                                            scalar1=sh_ml(k), scalar2=None,
                                            op0=mybir.AluOpType.add)
                    hi = xh2[k // 2][:, k % 2, :]
                    lo = xl2[k // 2][:, k % 2, :]
                    eh = nc.gpsimd if k % 2 == 0 else nc.vector
                    eh.tensor_copy(hi, xmb[k])
                    eh.tensor_sub(lo, xmb[k], hi)
                    nc.scalar.activation(out=x64[k // 2][:, k % 2, :],
                                         in_=xmb[k], func=AF.Identity,
                                         scale=1.0 / LOSC)

                mark("gate_up")
                # h2: fp8 DR-paired ffn activations
                h2 = [pca.tile([128, 2, LQ], F8, tag=f"h{t}", name=f"h2_{t}")
                      for t in range(FF // 256)]
                h64_2 = [pca.tile([128, 2, LQ], F8, tag=f"h6{t}", name=f"h64_{t}")
                         for t in range(FF // 256)]
                wdh_t, wdl_t = {}, {}

                def down_fetch(m):
                    wdh_t[m] = pca.tile([128, 16, 2, 128], F8, tag="wdw", bufs=4,
                                        name=f"wdh{m}")
                    dma(out=wdh_t[m], in_=wdh[m])
                    wdl_t[m] = pca.tile([128, 16, 2, 128], F8, tag="wdw", bufs=4,
                                        name=f"wdl{m}")
                    dma(out=wdl_t[m], in_=wdl[m])

                def dr_hilo(p1, wh, wl, mi, xlo=True):
                    # Xh*Wh + (X/64)*(Wl*64) [+ Xl*Wh], all at true scale
                    ms = slice(mi * 128, (mi + 1) * 128)
                    for kp in range(KP):
                        nc.tensor.matmul(p1, wh[:, kp, :, ms], xh2[kp][:, :, :],
                                         start=(kp == 0), stop=False, perf_mode=DR)
                    for kp in range(KP):
                        nc.tensor.matmul(p1, wl[:, kp, :, ms], x64[kp][:, :, :],
                                         start=False, stop=(not xlo and kp == KP - 1),
                                         perf_mode=DR)
                    if xlo:
                        for kp in range(KP):
                            nc.tensor.matmul(p1, wh[:, kp, :, ms], xl2[kp][:, :, :],
                                             start=False, stop=(kp == KP - 1),
                                             perf_mode=DR)

                for mg in range(8):
                    if mg >= 6:
                        down_fetch(mg - 6)
                    for mi in range(4):
                        pgu = ps.tile([128, 2 * LQ], FP32, tag="pQ", bufs=2,
                                      name=f"pgu{mg}_{mi}")
                        p1g = pgu[:, 0:LQ]
                        p1u = pgu[:, LQ:2 * LQ]
                        dr_hilo(p1g, wgh_t[mg], wgl_t[mg], mi)
                        sgl = pca.tile([128, LQ], BF16, tag="sgb", bufs=2,
                                       name=f"sgl{mg}_{mi}")
                        nc.scalar.activation(out=sgl, in_=p1g, func=AF.Silu)
                        dr_hilo(p1u, wuh_t[mg], wul_t[mg], mi)
                        t = mg * 4 + mi
                        nc.vector.tensor_mul(h2[t // 2][:, t % 2, :], sgl, p1u)
                        h64 = h64_2[t // 2][:, t % 2, :]
                        nc.scalar.activation(
                            out=h64, in_=h2[t // 2][:, t % 2, :],
                            func=AF.Identity, scale=1.0 / LOSC)

                mark("down")
                # down proj: P1 = H*Wdh + H64*Wdl64; out = P1*g + xres
                for m in range(CT):
                    if m + 2 < CT:
                        down_fetch(m + 2)
                    pd1 = ps.tile([128, LQ], FP32, tag="pA", bufs=2, name=f"pd1{m}")
                    for fp in range(16):
                        nc.tensor.matmul(pd1, wdh_t[m][:, fp, :, :],
                                         h2[fp][:, :, :],
                                         start=(fp == 0), stop=False,
                                         perf_mode=DR)
                    for fp in range(16):
                        nc.tensor.matmul(pd1, wdl_t[m][:, fp, :, :],
                                         h64_2[fp][:, :, :],
                                         start=False, stop=(fp == 15),
                                         perf_mode=DR)
                    of = pca.tile([128, LQ], FP32, tag="of", bufs=2, name=f"of{m}")
                    nc.vector.scalar_tensor_tensor(
                        out=of, in0=pd1, scalar=g_ml(m), in1=xres[:, m, :],
                        op0=mybir.AluOpType.mult, op1=mybir.AluOpType.add)
                    dma(out=outT[m * 128:(m + 1) * 128, :], in_=of)

    nc.compile()
    return nc


def _dr_perm(rope: bool):
    # column order for the (hg, j) DR layout: new col o=(hg*2+j)*128+p
    # takes original W column idx[o].
    idx = np.zeros(C, dtype=np.int64)
    for hg in range(4):
        for j in range(2):
            for p in range(128):
                h = hg * 4 + p // 32
                d = j * 32 + p % 32
                if rope:
                    # rotate-half pair i=(d%32): real=2i, imag=2i+1
                    c0 = h * 64 + 2 * (d % 32) + (0 if d < 32 else 1)
                else:
                    c0 = h * 64 + d
                idx[(hg * 2 + j) * 128 + p] = c0
    return idx


def _bf(a):
    return np.ascontiguousarray(a).astype(ml_dtypes.bfloat16)


def _f8(a):
    return np.ascontiguousarray(a).astype(F8NP)


def _dr_pack(W):
    # [n_in, n_out] -> [128, n_in//256, 2, n_out]
    n_in, n_out = W.shape
    kp = n_in // 256
    return W.reshape(kp, 2, 128, n_out).transpose(2, 0, 1, 3)


def _hilo(W):
    hi = W.astype(F8NP)
    lo = ((W - hi.astype(np.float32)) * LOSC).astype(F8NP)
    return hi, lo


def _prep_shared(W_qkv, W_sa_out, W_q, W_kv, W_ca_out, W_gate, W_up, W_down,
                 adaLN_W, adaLN_b, norm1_w, norm2_w, norm3_w):
    idx_r = _dr_perm(True)
    idx_n = _dr_perm(False)
    wq = W_qkv[:, 0:C][:, idx_r]
    wk = W_qkv[:, C:2 * C][:, idx_r]
    wv = W_qkv[:, 2 * C:3 * C]
    wqc = np.asarray(W_q, np.float32)[:, idx_n]
    wkv = np.concatenate([np.asarray(W_kv, np.float32)[:, 0:C][:, idx_n],
                          np.asarray(W_kv, np.float32)[:, C:2 * C]], axis=1)

    def pack8(W):
        return _f8(_dr_pack(np.asarray(W, np.float32)))

    wgh_, wgl_ = _hilo(np.asarray(W_gate, np.float32))
    wuh_, wul_ = _hilo(np.asarray(W_up, np.float32))
    wdh_, wdl_ = _hilo(np.asarray(W_down, np.float32))

    def mlp_pack(w8):  # fp8 [C, FF] -> [8 mg][128, kp, 2, 512]
        d = _dr_pack(w8.astype(np.float32)).astype(F8NP)
        return np.ascontiguousarray(d.reshape(128, KP, 2, 8, 512)
                                    .transpose(3, 0, 1, 2, 4))

    def down_pack(w8):  # fp8 [FF, C] -> [8 m][128, 16 fp, 2, 128]
        d = _dr_pack(w8.astype(np.float32)).astype(F8NP)
        return np.ascontiguousarray(d.reshape(128, 16, 2, CT, 128)
                                    .transpose(3, 0, 1, 2, 4))

    wada = np.asarray(adaLN_W, np.float32).reshape(CT, 128, 48, 128)
    wadaA_h = wada[:, :, 0:16, :].transpose(1, 0, 2, 3).reshape(128, CT, 2048)
    wadaB_h = np.stack([
        wada[:, :, 16 + 4 * i:20 + 4 * i, :].transpose(1, 0, 2, 3)
        .reshape(128, CT, 512) for i in range(8)])

    sh = {
        "wq2": pack8(wq), "wk2": pack8(wk), "wv2": pack8(wv),
        "wsa2": pack8(W_sa_out), "wqc2": pack8(wqc), "wkv2": pack8(wkv),
        "wca2": pack8(W_ca_out),
        "wgh": mlp_pack(wgh_), "wgl": mlp_pack(wgl_),
        "wuh": mlp_pack(wuh_), "wul": mlp_pack(wul_),
        "wdh": down_pack(wdh_), "wdl": down_pack(wdl_),
        "wadaA": _f8(wadaA_h), "wadaB": _bf(wadaB_h),
        "cst_base": np.concatenate([
            np.asarray(adaLN_b, np.float32).reshape(48, 128).T,
            np.asarray(norm1_w, np.float32).reshape(8, 128).T,
            np.asarray(norm2_w, np.float32).reshape(8, 128).T,
            np.asarray(norm3_w, np.float32).reshape(8, 128).T], axis=1),
    }
    return sh


def make_in_maps(x, t_mod, audio_context, freqs_cos, freqs_sin,
                 norm1_w, norm2_w, norm3_w,
                 W_qkv, W_sa_out, W_q, W_kv, W_ca_out,
                 W_gate, W_up, W_down, adaLN_W, adaLN_b):
    sh = _prep_shared(W_qkv, W_sa_out, W_q, W_kv, W_ca_out, W_gate, W_up,
                      W_down, adaLN_W, adaLN_b, norm1_w, norm2_w, norm3_w)
    cosT = np.ascontiguousarray(np.asarray(freqs_cos, np.float32).T)
    sinT = np.ascontiguousarray(np.asarray(freqs_sin, np.float32).T)

    in_maps = []
    for core in range(NCORE):
        b, j = divmod(core, 4)
        # roll the token axis so this core's own 512 tokens sit at [0, LQ)
        xT = np.roll(np.ascontiguousarray(np.asarray(x, np.float32)[b].T),
                     -j * LQ, axis=1)
        m = {k: v for k, v in sh.items() if k != "cst_base"}
        m["x_bf"] = _bf(xT)
        m["xq_f"] = np.ascontiguousarray(xT[:, 0:LQ])
        cr = np.roll(cosT, -j * LQ, axis=1)
        sr = np.roll(sinT, -j * LQ, axis=1)
        m["cs4"] = _bf(np.concatenate([cr, cr, cr, cr], axis=0))
        m["ss4"] = _bf(np.concatenate([sr, sr, sr, sr], axis=0))
        m["aud2"] = _f8(_dr_pack(
            np.ascontiguousarray(np.asarray(audio_context, np.float32)[b].T)))
        m["cst"] = np.ascontiguousarray(np.concatenate(
            [np.asarray(t_mod, np.float32)[b].reshape(8, 128).T,
             sh["cst_base"]], axis=1))
        in_maps.append(m)
    return in_maps


_NC_CACHE = None


def _get_nc():
    global _NC_CACHE
    if _NC_CACHE is None:
        _NC_CACHE = build_bass()
    return _NC_CACHE


def kernel(**inputs):
    nc = _get_nc()
    inputs = {k: np.asarray(v) for k, v in inputs.items()}
    in_maps = make_in_maps(**inputs)
    res = run_bass_kernel_spmd(nc, in_maps, list(range(NCORE)))
    out = np.zeros((B, L, C), np.float32)
    for core in range(NCORE):
        b, j = divmod(core, 4)
        out[b, j * LQ:(j + 1) * LQ, :] = res.results[core]["outT"].T
    return out


# revision 71
# speedup vs baseline: 1.0118x; 1.0116x over previous
"""Trainium2 Bass kernel for nn_ExpressionModel (dense DiT-style transformer block).

Sharding: 8 cores = 2 (batch) x 4 (sequence chunks of 512 tokens).
Each core computes the full block for its 512 query tokens; K/V projections
for the full 2048-token batch are duplicated across the 4 cores of a batch
(no collectives needed).

Key layout trick (vs the previous revision): q/k projection output tiles are
(head-group hg of 4 heads, j) with j in {0,1} the rotate-half block of the
head dim; partition p = (h%4)*32 + d%32. The rope "swap" partner of
partition p in block j is partition p in block 1-j of the SAME tile pair --
no swapped-weight second projection and no partition shuffles. The j dim
doubles as the fp8 DoubleRow pairing dim, so:
  - scores run fp8-DR [32,2]-stationary (0.5 cyc/col vs 1.0 bf16). PE
    operand base partitions must be 0/32/64, so the 4th head of each group
    reads a base-64 [64,2] window against a shadow q tile (qz8/qcz8) whose
    sibling-head rows are zeroed.
  - the exp writes probs as fp8 [128, 2, LQ] and p@V runs one fp8-DR
    matmul per 256-key window (vsb [128, 2, H, D+1]) -- 4x cheaper than
    bf16 128-key chunks.
Engine budget: Act's exps are the hard floor (~1038ns per [128,1024]; SA
133us + CA 33us); everything else is spread over DVE/Pool/Act by measured
cost (Pool cannot touch PSUM, scalar_tensor_tensor is DVE-only). DMA is a
single serial ~360GB/s resource, so the preamble queue order is arrival-
priority: x0, consts, adaLN-A (fp8), cos/sin, w_q, x1, x2, w_k, x3, w_v.
k head-groups 2-3 and the audio cross-K/V stream inside the self-attn
exp stream; adaLN-B columns stream weight-stationary the same way.
The MLP keeps the hi+lo fp8 split (T ~ T_hi + T_lo/64) on gate/up (3-pass)
and down (2-pass) -- measured HW rel err 0.0138 of the 0.02 budget; 2-pass
gate/up variants model out to ~0.019 on HW, too close to ship. Silu is a
single Act op (table includes copy/identity so no thrash with h64 scaling).
"""

import numpy as np
import ml_dtypes

import concourse.bass as bass
import concourse.tile as tile
from concourse import bacc, mybir
from concourse.bass_utils import run_bass_kernel_spmd

FP32 = mybir.dt.float32
BF16 = mybir.dt.bfloat16
F8 = mybir.dt.float8e4
DR = mybir.MatmulPerfMode.DoubleRow
F8NP = ml_dtypes.float8_e4m3

STAGE_MARKS = []  # (instruction-id watermark, stage name) — profiling aid

B, L, C = 2, 2048, 1024
H, D = 16, 64
L2, TD = 512, 768
FF = 4096
EPS = 1e-6
NCORE = 8
LQ = 512            # query tokens per core
CT = C // 128       # 8 C partition-tiles
KP = C // 256       # 4 DoubleRow contraction pairs over C
LCH = L // 512      # 4 512-token chunks
KSC = 1.0 / 8.0     # 1/sqrt(D)
LOSC = 64.0         # hi/lo split scale
NW = L // 256       # 8 256-key windows (self attn)
NWC = L2 // 256     # 2 windows (cross attn)

AF = mybir.ActivationFunctionType


def build_bass():
    nc = bacc.Bacc("TRN2", target_bir_lowering=False, debug=False)
    STAGE_MARKS.clear()

    def mark(stage):
        STAGE_MARKS.append((nc.next_id(), stage))

    def dma(out, in_):
        return nc.sync.dma_start(out=out, in_=in_)

    def din(name, shape, dt):
        return nc.dram_tensor(name, list(shape), dt, kind="ExternalInput")

    # --- inputs ---
    x_bf = din("x_bf", (C, L), BF16)            # x[b].T, bf16
    xq_f = din("xq_f", (C, LQ), FP32)           # own-chunk x[b].T, fp32 residual
    aud2 = din("aud2", (128, 3, 2, L2), F8)     # audio.T fp8 DR-paired
    cst = din("cst", (128, 80), FP32)           # tmod|adab|n1|n2|n3
    cs4 = din("cs4", (128, L), BF16)            # cos[p%32] rows
    ss4 = din("ss4", (128, L), BF16)            # sin[p%32] rows (plain)
    wadaA = din("wadaA", (128, CT, 2048), F8)      # adaLN W cols j0..15
    wadaB = din("wadaB", (8, 128, CT, 512), BF16)  # adaLN W cols j16..47
    wq2 = din("wq2", (128, KP, 2, C), F8)       # W_qkv q, (hg,j) DR layout
    wk2 = din("wk2", (128, KP, 2, C), F8)
    wv2 = din("wv2", (128, KP, 2, C), F8)       # v natural
    wsa2 = din("wsa2", (128, KP, 2, C), F8)
    wqc2 = din("wqc2", (128, KP, 2, C), F8)     # cross q, (hg,j) layout
    wkv2 = din("wkv2", (128, 3, 2, 2 * C), F8)  # K half (hg,j), V natural
    wca2 = din("wca2", (128, KP, 2, C), F8)
    wgh = din("wgh", (8, 128, KP, 2, 512), F8)  # MLP weights hi/lo fp8
    wgl = din("wgl", (8, 128, KP, 2, 512), F8)
    wuh = din("wuh", (8, 128, KP, 2, 512), F8)
    wul = din("wul", (8, 128, KP, 2, 512), F8)
    wdh = din("wdh", (CT, 128, 16, 2, 128), F8)
    wdl = din("wdl", (CT, 128, 16, 2, 128), F8)

    outT = nc.dram_tensor("outT", [C, LQ], FP32, kind="ExternalOutput")

    with tile.TileContext(nc) as tc:
        with (
            tc.tile_pool(name="pp", bufs=1) as pp,              # persistent
            tc.tile_pool(name="ps", bufs=1, space="PSUM") as ps,
        ):
            # ---- persistent constants (one packed tile) ----
            c_all = pp.tile([128, 80], FP32, tag="c_all")
            c_tmod = c_all[:, 0:CT]
            c_adab = c_all[:, 8:56]
            c_n1 = c_all[:, 56:64]
            c_n2 = c_all[:, 64:72]
            c_n3 = c_all[:, 72:80]
            c_cs4 = pp.tile([128, L], BF16, tag="c_cs4")
            c_ss4 = pp.tile([128, L], BF16, tag="c_ss4")
            xres = pp.tile([128, CT, LQ], FP32, tag="xres")
            ones_col = pp.tile([128, 1], BF16, tag="ones_col")
            ones_row = pp.tile([1, 128], BF16, tag="ones_row")
            eps_c = pp.tile([1, 1], FP32, tag="eps_c")
            nc.gpsimd.memset(ones_col, 1.0)
            nc.gpsimd.memset(ones_row, 1.0)
            nc.gpsimd.memset(eps_c, EPS)
            modsT = pp.tile([128, 48], FP32, tag="modsT")
            silu_bf = pp.tile([128, CT], BF16, tag="silu_bf")
            silu_f8 = pp.tile([128, CT], F8, tag="silu_f8")
            w1eff = pp.tile([128, CT], FP32, tag="w1eff")
            w3eff = pp.tile([128, CT], FP32, tag="w3eff")
            # attn output accumulators (fp8, DR-paired; reused by cross attn)
            att2 = [pp.tile([128, 2, LQ], F8, tag=f"att{j}", name=f"att{j}")
                    for j in range(KP)]
            # cross K (hg,j layout) / V (natural, 256-key windows)
            kcT8 = [pp.tile([128, 2, L2], F8, tag=f"kc{g}", name=f"kcT{g}")
                    for g in range(4)]
            vcb8 = [pp.tile([128, 2, H, D + 1], F8, tag=f"vc{w}",
                            name=f"vcb{w}") for w in range(NWC)]

            def sh_sa(k):
                return modsT[:, 0 + k:1 + k]

            def g_sa(k):
                return modsT[:, 16 + k:17 + k]

            def sh_ml(k):
                return modsT[:, 24 + k:25 + k]

            def g_ml(k):
                return modsT[:, 40 + k:41 + k]

            with tc.tile_pool(name="pkv", bufs=1) as pkv:
                # V in 256-key windows: [128, j, head, D+1] fp8
                vsb8 = [pkv.tile([128, 2, H, D + 1], F8, tag=f"v{w}",
                                 name=f"v{w}") for w in range(NW)]
                qT8 = [pkv.tile([128, 2, LQ], F8, tag=f"qT{g}", name=f"qT{g}")
                       for g in range(4)]
                # head3 shadow: base-64 matmul window [64:128) with head2's
                # rows zeroed (PE ops only allow base partition 0/32/64)
                qz8 = [pkv.tile([128, 2, LQ], F8, tag=f"qz{g}", name=f"qz{g}")
                       for g in range(4)]
                # adaLN-A weights borrow the kT8 slots (unused until k_proj)
                # allocated in fetch order: scale_sa pieces (4..7) first
                ADA_ORD = [4, 5, 6, 7, 0, 1, 2, 3]
                wadaA_t = {}
                for i in ADA_ORD:
                    wadaA_t[i] = pkv.tile([128, L], F8, tag="wadaAx", bufs=6,
                                          name=f"wadaA{i}")
                # SA-phase tiles (former pat pool, merged so k_proj can
                # overlap the SA stream)
                w_sa = pkv.tile([128, KP, 2, C], F8, tag="w_sa")
                w_kv = pkv.tile([128, 3, 2, 2 * C], F8, tag="w_kv")
                a_t = pkv.tile([128, 3, 2, L2], F8, tag="a_t")
                wadaB_t = {}

                with tc.tile_pool(name="pqw", bufs=1) as pqw:
                    w_q = pqw.tile([128, KP, 2, C], F8, tag="wmain", bufs=2,
                                   name="w_q")
                    xsa2 = [[pqw.tile([128, 2, 512], F8, tag=f"xsa{j}_{lc}",
                                      name=f"xsa{j}_{lc}")
                             for j in range(KP)] for lc in range(LCH)]
                    xc = {}

                    def x_fetch(lc, q=None):
                        halves = []
                        for hf in range(2):
                            t = pqw.tile([128, CT // 2, 512], BF16, tag="xinc",
                                         bufs=6, name=f"xin{lc}_{hf}")
                            (q or nc.sync).dma_start(
                                out=t, in_=x_bf[:, :].rearrange(
                                    "(k p) l -> p k l", p=128)
                                [:, hf * 4:(hf + 1) * 4,
                                 lc * 512:(lc + 1) * 512])
                            halves.append(t)
                        xc[lc] = halves

                    def xck(lc, k):
                        return xc[lc][k // 4][:, k % 4, :]

                    # ---- DMA issue order (SP FIFO) ----
                    x_fetch(0)
                    dma(out=c_all, in_=cst[:, :])
                    for i in (4, 5, 6, 7):
                        dma(out=wadaA_t[i],
                            in_=wadaA[:, :, i * 256:(i + 1) * 256])
                    dma(out=c_cs4, in_=cs4[:, :])
                    dma(out=c_ss4, in_=ss4[:, :])
                    dma(out=w_q, in_=wq2[:, :, :, :])
                    for i in (0, 1, 2, 3):
                        dma(out=wadaA_t[i],
                            in_=wadaA[:, :, i * 256:(i + 1) * 256])
                    x_fetch(1)
                    x_fetch(2)

                    mark("norm1")
                    # ---- silu(t_mod) on Act directly ----
                    nc.scalar.activation(out=silu_bf, in_=c_tmod, func=AF.Silu)
                    nc.vector.tensor_copy(silu_f8, silu_bf)

                    pbs = {}

                    def norm1_ssq(lc):
                        pssq = ps.tile([1, 512], FP32, tag="pC", bufs=2,
                                       name=f"pssq{lc}")
                        for k in range(CT):
                            xsq = pqw.tile([128, 512], BF16, tag="xsq", bufs=1,
                                           name=f"xsq{lc}_{k}")
                            if k % 2 == 0:
                                nc.vector.tensor_mul(xsq, xck(lc, k),
                                                     xck(lc, k))
                            else:
                                nc.scalar.activation(out=xsq,
                                                     in_=xck(lc, k),
                                                     func=AF.Square)
                            nc.tensor.matmul(pssq, ones_col, xsq,
                                             start=(k == 0), stop=(k == CT - 1))
                        rstd = pqw.tile([1, 512], FP32, tag="rstd", bufs=2,
                                        name=f"rstd{lc}")
                        nc.scalar.activation(out=rstd, in_=pssq, func=AF.Sqrt,
                                             bias=eps_c, scale=1.0 / C)
                        rstd_bf = pqw.tile([1, 512], BF16, tag="rstd_bf", bufs=2,
                                           name=f"rstdb{lc}")
                        with nc.allow_low_precision(reason="rstd bf16, matches prior fp32-recip+bf16-copy"):
                            nc.vector.reciprocal(rstd_bf, rstd)
                        pb = ps.tile([128, 512], FP32, tag="pA", bufs=2,
                                     name=f"pbn1{lc}")
                        nc.tensor.matmul(pb, ones_row, rstd_bf, start=True, stop=True)
                        pbsb = pqw.tile([128, 512], BF16, tag="pbsb", bufs=2,
                                        name=f"pbsb{lc}")
                        nc.scalar.copy(out=pbsb, in_=pb)
                        pbs[lc] = pbsb

                    def mod1(lc):
                        for k in range(CT):
                            dst = xsa2[lc][k // 2][:, k % 2, :]
                            eng = nc.vector
                            eng.scalar_tensor_tensor(
                                out=dst, in0=xck(lc, k),
                                scalar=w1eff[:, k:k + 1], in1=pbs[lc],
                                op0=mybir.AluOpType.mult,
                                op1=mybir.AluOpType.mult)
                            if k % 2 == 0:
                                nc.scalar.activation(
                                    out=dst, in_=dst, func=AF.Identity,
                                    bias=sh_sa(k))
                            else:
                                nc.gpsimd.tensor_scalar(
                                    out=dst, in0=dst, scalar1=sh_sa(k),
                                    scalar2=None, op0=mybir.AluOpType.add)

                    norm1_ssq(0)

                    mark("adaLN")
                    # ---- adaLN part A: scale_sa first (w1eff path), then shift ----
                    pmA = ps.tile([128, 16], FP32, tag="pC", bufs=2, name="pmA")
                    for j in list(range(8, 16)) + list(range(8)):
                        for k in range(CT):
                            nc.tensor.matmul(pmA[:, j:j + 1],
                                             wadaA_t[j // 2][:, k * 256 + (j % 2) * 128:
                                                             k * 256 + (j % 2) * 128 + 128],
                                             silu_f8[:, k:k + 1],
                                             start=(k == 0), stop=(k == CT - 1))
                        if j == 15:
                            nc.vector.tensor_add(modsT[:, 8:16], pmA[:, 8:16],
                                                 c_adab[:, 8:16])
                            nc.vector.tensor_scalar(out=w1eff, in0=modsT[:, 8:16],
                                                    scalar1=1.0, scalar2=None,
                                                    op0=mybir.AluOpType.add)
                            nc.vector.tensor_mul(w1eff, w1eff, c_n1)
                    nc.vector.tensor_add(modsT[:, 0:8], pmA[:, 0:8],
                                         c_adab[:, 0:8])

                    mark("mod1")
                    mod1(0)
                    norm1_ssq(1)
                    w_k = pqw.tile([128, KP, 2, C], F8, tag="wmain", bufs=2,
                                   name="w_k")
                    w_v = pqw.tile([128, KP, 2, C], F8, tag="wmain", bufs=2,
                                   name="w_v")
                    dma(out=w_k, in_=wk2[:, :, :, :])
                    x_fetch(3)
                    dma(out=w_v, in_=wv2[:, :, :, :])

                    def proj_dr(out_psum, w, m, xcols, nkp=KP):
                        for kp in range(nkp):
                            nc.tensor.matmul(out_psum,
                                             w[:, kp, :, m * 128:(m + 1) * 128],
                                             xcols(kp),
                                             start=(kp == 0), stop=(kp == nkp - 1),
                                             perf_mode=DR)

                    kT8 = [pkv.tile([128, 2, L], F8, tag="kTx", bufs=4,
                                    name=f"kT{g}") for g in range(4)]
                    rope_rr = [0]

                    def rope_unit2(dst, pk0, pk1, cols, act_ok=True):
                        kb0 = pkv.tile([128, 512], BF16, tag="ropet", bufs=8,
                                       name="kb0")
                        nc.vector.tensor_copy(kb0, pk0)
                        kb1 = pkv.tile([128, 512], BF16, tag="ropet", bufs=8,
                                       name="kb1")
                        if act_ok:
                            nc.scalar.copy(out=kb1, in_=pk1)
                        else:
                            nc.vector.tensor_copy(kb1, pk1)
                        ma = pkv.tile([128, 512], BF16, tag="ropet", bufs=8,
                                      name="ma")
                        nc.vector.tensor_mul(ma, kb0, c_cs4[:, cols])
                        mb = pkv.tile([128, 512], BF16, tag="ropet", bufs=8,
                                      name="mb")
                        nc.vector.tensor_mul(mb, kb1, c_ss4[:, cols])
                        mc = pkv.tile([128, 512], BF16, tag="ropet", bufs=8,
                                      name="mc")
                        nc.vector.tensor_mul(mc, kb0, c_ss4[:, cols])
                        md = pkv.tile([128, 512], BF16, tag="ropet", bufs=8,
                                      name="md")
                        nc.vector.tensor_mul(md, kb1, c_cs4[:, cols])
                        rope_rr[0] ^= 1
                        if rope_rr[0]:
                            nc.vector.tensor_sub(dst[:, 0, cols], ma, mb)
                            nc.gpsimd.tensor_add(dst[:, 1, cols], mc, md)
                        else:
                            nc.gpsimd.tensor_sub(dst[:, 0, cols], ma, mb)
                            nc.vector.tensor_add(dst[:, 1, cols], mc, md)

                    def rope_unit(dst, pq2, cols):
                        # dst[:,0,cols] = pq2[:,0]*cos - pq2[:,1]*sin
                        # dst[:,1,cols] = pq2[:,0]*sin + pq2[:,1]*cos
                        kb0 = pkv.tile([128, 512], BF16, tag="ropet", bufs=8,
                                       name="kb0")
                        nc.vector.tensor_copy(kb0, pq2[:, 0, :])
                        kb1 = pkv.tile([128, 512], BF16, tag="ropet", bufs=8,
                                       name="kb1")
                        nc.scalar.copy(out=kb1, in_=pq2[:, 1, :])
                        ma = pkv.tile([128, 512], BF16, tag="ropet", bufs=8,
                                      name="ma")
                        nc.vector.tensor_mul(ma, kb0, c_cs4[:, cols])
                        mb = pkv.tile([128, 512], BF16, tag="ropet", bufs=8,
                                      name="mb")
                        nc.vector.tensor_mul(mb, kb1, c_ss4[:, cols])
                        mc = pkv.tile([128, 512], BF16, tag="ropet", bufs=8,
                                      name="mc")
                        nc.vector.tensor_mul(mc, kb0, c_ss4[:, cols])
                        md = pkv.tile([128, 512], BF16, tag="ropet", bufs=8,
                                      name="md")
                        nc.vector.tensor_mul(md, kb1, c_cs4[:, cols])
                        # final adds alternate DVE / Pool to balance load
                        rope_rr[0] ^= 1
                        if rope_rr[0]:
                            nc.vector.tensor_sub(dst[:, 0, cols], ma, mb)
                            nc.gpsimd.tensor_add(dst[:, 1, cols], mc, md)
                        else:
                            nc.gpsimd.tensor_sub(dst[:, 0, cols], ma, mb)
                            nc.vector.tensor_add(dst[:, 1, cols], mc, md)

                    mark("q_proj")
                    # ====== q projection (own chunk = mod chunk 0) + rope ======
                    OWN = slice(0, LQ)
                    for hg in range(4):
                        pq2 = ps.tile([128, 2, LQ], FP32, tag="pQ", bufs=2,
                                      name=f"pq{hg}")
                        proj_dr(pq2[:, 0, :], w_q, hg * 2,
                                lambda kp: xsa2[0][kp][:, :, :])
                        proj_dr(pq2[:, 1, :], w_q, hg * 2 + 1,
                                lambda kp: xsa2[0][kp][:, :, :])
                        rope_unit(qT8[hg], pq2, OWN)
                        nc.vector.memset(qz8[hg][64:96, :, :], 0.0)
                        nc.scalar.copy(out=qz8[hg][96:128, :, :],
                                       in_=qT8[hg][96:128, :, :])
                        if hg == 0:
                            norm1_ssq(2)
                            mod1(1)
                        if hg == 1:
                            mod1(2)
                        if hg == 2:
                            norm1_ssq(3)
                        if hg == 3:
                            mod1(3)

                    mark("k_proj")
                    # ====== k projection + rope (hg0 now, hg1-3 in SA stream);
                    #        v units interleaved ======
                    dma(out=xres, in_=xq_f[:, :].rearrange(
                        "(k p) l -> p k l", p=128))
                    dma(out=w_kv, in_=wkv2[:, :, :, :])
                    dma(out=a_t, in_=aud2[:, :, :, :])
                    dma(out=w_sa, in_=wsa2[:, :, :, :])

                    def k_unit(hg, lc):
                        sl = slice(lc * 512, (lc + 1) * 512)
                        pka = ps.tile([128, 512], FP32, tag="pA", bufs=2,
                                      name=f"pk{hg}_{lc}a")
                        pkb = ps.tile([128, 512], FP32, tag="pA", bufs=2,
                                      name=f"pk{hg}_{lc}b")
                        proj_dr(pka, w_k, hg * 2,
                                lambda kp: xsa2[lc][kp][:, :, :])
                        proj_dr(pkb, w_k, hg * 2 + 1,
                                lambda kp: xsa2[lc][kp][:, :, :])
                        rope_unit2(kT8[hg], pka, pkb, sl,
                                   act_ok=(hg < 2))

                    def v_unit(t, g):
                        # t: 128-token chunk 0..15, g: channel half
                        w = t // 2
                        if g == 0 and t % 2 == 0:
                            nc.vector.memset(vsb8[w][:, :, :, D:D + 1], 1.0)
                        pv = ps.tile([128, 512], FP32, tag="pA", bufs=2,
                                     name=f"pv{t}_{g}")
                        for kp in range(KP):
                            nc.tensor.matmul(
                                pv, xsa2[t // 4][kp][:, :, (t % 4) * 128:
                                                     (t % 4) * 128 + 128],
                                w_v[:, kp, :, g * 512:(g + 1) * 512],
                                start=(kp == 0), stop=(kp == KP - 1),
                                perf_mode=DR)
                        dstv = vsb8[w][:, t % 2, g * 8:(g + 1) * 8, 0:D]
                        srcv = pv.rearrange("p (h d) -> p h d", h=8)
                        if t < 8:
                            nc.scalar.copy(out=dstv, in_=srcv)
                        else:
                            nc.vector.tensor_copy(dstv, srcv)

                    for lc in range(LCH):
                        k_unit(0, lc)
                        for t in range(lc * 2, lc * 2 + 2):
                            v_unit(t, 0)
                            v_unit(t, 1)
                    mark("v_proj")
                    for lc in range(LCH):
                        k_unit(1, lc)
                        for t in range(8 + lc * 2, 8 + lc * 2 + 2):
                            v_unit(t, 0)
                            v_unit(t, 1)

                    # ---- SA-phase helpers (cross kv, adaLN-B) ----
                    def adaB_fetch(i):
                        for hf in range(2):
                            wadaB_t[(i, hf)] = pkv.tile(
                                [128, CT // 2, 512], BF16, tag="wadaB",
                                bufs=2, name=f"wadaB{i}_{hf}")
                            dma(out=wadaB_t[(i, hf)],
                                in_=wadaB[i, :, hf * 4:(hf + 1) * 4, :])

                    def cross_kv_piece(i):
                        # i 0..7: kc tile (hg, j); i 8..15: vc (tchunk, ghalf)
                        if i < 8:
                            hg, j = divmod(i, 2)
                            pkc = ps.tile([128, L2], FP32, tag="pA", bufs=2,
                                          name=f"pkc{i}")
                            for kp in range(3):
                                nc.tensor.matmul(pkc,
                                                 w_kv[:, kp, :, i * 128:(i + 1) * 128],
                                                 a_t[:, kp, :, :],
                                                 start=(kp == 0), stop=(kp == 2),
                                                 perf_mode=DR)
                            nc.vector.tensor_copy(kcT8[hg][:, j, :], pkc)
                        else:
                            t, g = divmod(i - 8, 2)
                            if g == 0 and t % 2 == 0:
                                nc.vector.memset(
                                    vcb8[t // 2][:, :, :, D:D + 1], 1.0)
                            pvc = ps.tile([128, 512], FP32, tag="pA", bufs=2,
                                          name=f"pvc{i}")
                            for kp in range(3):
                                nc.tensor.matmul(
                                    pvc, a_t[:, kp, :, t * 128:(t + 1) * 128],
                                    w_kv[:, kp, :, C + g * 512:C + (g + 1) * 512],
                                    start=(kp == 0), stop=(kp == 2),
                                    perf_mode=DR)
                            nc.vector.tensor_copy(
                                vcb8[t // 2][:, t % 2, g * 8:(g + 1) * 8, 0:D],
                                pvc.rearrange("p (h d) -> p h d", h=8))

                    def adaB_piece(i):
                        j0 = 16 + 4 * i
                        pmB = ps.tile([128, 4], FP32, tag="pA", bufs=2,
                                      name=f"pmB{i}")
                        for jj in range(4):
                            for k in range(CT):
                                nc.tensor.matmul(pmB[:, jj:jj + 1],
                                                 wadaB_t[(i, k // 4)][:, k % 4,
                                                                      jj * 128:(jj + 1) * 128],
                                                 silu_bf[:, k:k + 1],
                                                 start=(k == 0), stop=(k == CT - 1))
                        nc.vector.tensor_add(modsT[:, j0:j0 + 4], pmB,
                                             c_adab[:, j0:j0 + 4])
                        if i == 5:
                            nc.vector.tensor_scalar(out=w3eff, in0=modsT[:, 32:40],
                                                    scalar1=1.0, scalar2=None,
                                                    op0=mybir.AluOpType.add)
                            nc.vector.tensor_mul(w3eff, w3eff, c_n3)

                    def proj_dr2(out_psum, w, m, xcols, nkp=KP):
                        for kp in range(nkp):
                            nc.tensor.matmul(out_psum,
                                             w[:, kp, :, m * 128:(m + 1) * 128],
                                             xcols(kp),
                                             start=(kp == 0),
                                             stop=(kp == nkp - 1),
                                             perf_mode=DR)

                    def sa_out_unit(m):
                        pso = ps.tile([128, LQ], FP32, tag="pA", bufs=2,
                                      name=f"pso{m}")
                        proj_dr2(pso, w_sa, m, lambda kp: att2[kp][:, :, :])
                        nc.vector.scalar_tensor_tensor(
                            out=xres[:, m, :], in0=pso, scalar=g_sa(m),
                            in1=xres[:, m, :],
                            op0=mybir.AluOpType.mult, op1=mybir.AluOpType.add)

                    mark("self_attn")
                    # ====== self-attention stream: item g = (h, w256) ======
                    LAG = 3
                    pos = {}
                    pexps = {}
                    pending = []  # (due_item, closure) in issue order

                    def sa_epilogue(h):
                        def run():
                            m = h // 2
                            rs = slice((h % 2) * 64, (h % 2) * 64 + 64)
                            po = pos.pop(h)
                            rec_bf = pp.tile([1, LQ], BF16, tag="rec_bf", bufs=2,
                                             name=f"recb{h}")
                            with nc.allow_low_precision(reason="softmax 1/sum bf16"):
                                nc.vector.reciprocal(rec_bf, po[64:65, :])
                            pbc = ps.tile([64, LQ], FP32, tag="pA", bufs=2,
                                          name=f"pbc{h}")
                            nc.tensor.matmul(pbc, ones_row[:, 0:64], rec_bf,
                                             start=True, stop=True)
                            rb_sb = pp.tile([64, LQ], BF16, tag="rb_sb", bufs=2,
                                            name=f"rb{h}")
                            nc.vector.tensor_copy(rb_sb, pbc)
                            nc.vector.tensor_mul(att2[m // 2][rs, m % 2, :],
                                                 po[0:64, :], rb_sb)
                        return run

                    def sa_po(h, w):
                        def run():
                            px = pexps.pop((h, w))
                            nc.tensor.matmul(pos[h], vsb8[w][:, :, h, :], px,
                                             start=(w == 0), stop=(w == NW - 1),
                                             perf_mode=DR)
                        return run

                    NIT = H * NW
                    for g in range(NIT + NW):
                        while pending and pending[0][0] <= g:
                            pending.pop(0)[1]()
                        if g >= NIT:
                            continue
                        h, w = divmod(g, NW)
                        hg = h // 4
                        rs = slice((h % 4) * 32, (h % 4) * 32 + 32)
                        if w == 0:
                            pos[h] = ps.tile([65, LQ], FP32, tag="pC", bufs=2,
                                             name=f"po{h}")
                            if h < 8:
                                adaB_fetch(h)
                        psc = ps.tile([128, 2, LQ], FP32, tag="pQ", bufs=2,
                                      name=f"psc{h}_{w}")
                        if h % 4 == 3:
                            rs = slice(64, 128)
                            qmov = qz8[hg]
                        else:
                            qmov = qT8[hg]
                        for jj in (0, 1):
                            t = 2 * w + jj
                            nc.tensor.matmul(psc[:, jj, :],
                                             kT8[hg][rs, :, t * 128:(t + 1) * 128],
                                             qmov[rs, :, :],
                                             start=True, stop=True,
                                             perf_mode=DR)
                        pexp = pkv.tile([128, 2, LQ], F8, tag="pexpS", bufs=7,
                                        name=f"pexp{h}_{w}")
                        nc.scalar.activation(out=pexp, in_=psc, func=AF.Exp,
                                             scale=KSC)
                        pexps[(h, w)] = pexp
                        pending.append((g + LAG, sa_po(h, w)))
                        # interleaved work: late k units, cross kv, adaLN-B
                        if w == 6 and h < 8:
                            k_unit(2 + h // 4, h % 4)
                        if w == NW - 1:
                            pending.append((g + LAG + 2, sa_epilogue(h)))
                            if 2 <= h < 10:
                                pending.append((g + LAG + 3, (lambda hh:
                                    lambda: cross_kv_piece(2 * (hh - 2)))(h)))
                                pending.append((g + LAG + 3, (lambda hh:
                                    lambda: cross_kv_piece(2 * (hh - 2) + 1))(h)))
                            if h >= 8:
                                pending.append((g + LAG + 4, (lambda hh:
                                    lambda: adaB_piece(hh - 8))(h)))
                    while pending:
                        pending.pop(0)[1]()

                mark("sa_out")
                # ====== sa_out (needs ALL heads' att2) + norm2 ssq (lagged) ======
                pssq_n2 = ps.tile([1, LQ], FP32, tag="pC", bufs=2, name="pssq_n2")

                def n2_ssq(m):
                    xsq = pp.tile([128, LQ], BF16, tag="rb_sb", bufs=2,
                                  name=f"xsqn2_{m}")
                    nc.scalar.activation(out=xsq, in_=xres[:, m, :],
                                         func=AF.Square)
                    nc.tensor.matmul(pssq_n2, ones_col, xsq,
                                     start=(m == 0), stop=(m == CT - 1))

                for m in range(CT):
                    sa_out_unit(m)
                for m in range(CT):
                    n2_ssq(m)

            mark("cross")
            # ====== cross attention + MLP ======
            with tc.tile_pool(name="pca", bufs=1) as pca:
                w_qc = pca.tile([128, KP, 2, C], F8, tag="w_qc")
                dma(out=w_qc, in_=wqc2[:, :, :, :])
                w_ca = pca.tile([128, KP, 2, C], F8, tag="w_ca")
                dma(out=w_ca, in_=wca2[:, :, :, :])
                wgh_t, wgl_t, wuh_t, wul_t = {}, {}, {}, {}

                def gu_fetch(mg):
                    for dd, src_, nm in ((wgh_t, wgh, "gh"), (wgl_t, wgl, "gl"),
                                         (wuh_t, wuh, "uh"), (wul_t, wul, "ul")):
                        dd[mg] = pca.tile([128, KP, 2, 512], F8, tag="wgu", bufs=8,
                                          name=f"w{nm}{mg}")
                        dma(out=dd[mg], in_=src_[mg])

                gu_fetch(0)
                gu_fetch(1)

                # norm2 (no modulation) -> xnb2 fp8 DR-paired
                xnb2 = [pca.tile([128, 2, LQ], F8, tag=f"xn{j}", name=f"xnb{j}")
                        for j in range(KP)]
                rstd2 = pca.tile([1, LQ], FP32, tag="rstd", bufs=2,
                                 name="rstd_n2")
                nc.scalar.activation(out=rstd2, in_=pssq_n2, func=AF.Sqrt,
                                     bias=eps_c, scale=1.0 / C)
                rstd2_bf = pca.tile([1, LQ], BF16, tag="rstd_bf", bufs=2,
                                    name="rstdb_n2")
                with nc.allow_low_precision(reason="rstd bf16"):
                    nc.vector.reciprocal(rstd2_bf, rstd2)
                pb2 = ps.tile([128, LQ], FP32, tag="pC", bufs=2, name="pb_n2")
                nc.tensor.matmul(pb2, ones_row, rstd2_bf, start=True, stop=True)
                pb2sb = pca.tile([128, LQ], BF16, tag="pb2sb", bufs=1,
                                 name="pb2sb")
                nc.scalar.copy(out=pb2sb, in_=pb2)
                for k in range(CT):
                    eng = nc.vector
                    eng.scalar_tensor_tensor(
                        out=xnb2[k // 2][:, k % 2, :], in0=xres[:, k, :],
                        scalar=c_n2[:, k:k + 1], in1=pb2sb,
                        op0=mybir.AluOpType.mult, op1=mybir.AluOpType.mult)

                # cross q projection into (hg, j) fp8 layout
                qcT8 = [pca.tile([128, 2, LQ], F8, tag=f"qc{g}", name=f"qcT{g}")
                        for g in range(4)]
                qcz8 = [pca.tile([128, 2, LQ], F8, tag=f"qcz{g}", name=f"qcz{g}")
                        for g in range(4)]

                def qc_unit(hg, j):
                    pq = ps.tile([128, LQ], FP32, tag="pA", bufs=2,
                                 name=f"pqc{hg}_{j}")
                    proj_dr2(pq, w_qc, hg * 2 + j, lambda kp: xnb2[kp][:, :, :])
                    nc.scalar.copy(out=qcT8[hg][:, j, :], in_=pq)
                    if j == 0:
                        nc.vector.memset(qcz8[hg][64:96, :, :], 0.0)
                    nc.scalar.copy(out=qcz8[hg][96:128, j, :],
                                   in_=pq[96:128, :])

                for hg, j in ((0, 0), (0, 1), (1, 0), (1, 1)):
                    qc_unit(hg, j)

                def ca_out_unit(m):
                    pco = ps.tile([128, LQ], FP32, tag="pA", bufs=2,
                                  name=f"pcao{m}")
                    proj_dr2(pco, w_ca, m, lambda kp: att2[kp][:, :, :])
                    nc.vector.tensor_add(xres[:, m, :], xres[:, m, :], pco)

                mark("cross_attn")
                # ====== cross-attention stream: item g = (h, w256) ======
                CLAG = 2
                pos = {}
                pexps = {}
                pending = []

                def ca_epilogue(h):
                    def run():
                        m = h // 2
                        rs = slice((h % 2) * 64, (h % 2) * 64 + 64)
                        po = pos.pop(h)
                        rec_bf = pp.tile([1, LQ], BF16, tag="rec_bf", bufs=2,
                                         name=f"recbc{h}")
                        with nc.allow_low_precision(reason="softmax 1/sum bf16"):
                            nc.vector.reciprocal(rec_bf, po[64:65, :])
                        pbc = ps.tile([64, LQ], FP32, tag="pA", bufs=2,
                                      name=f"pbcc{h}")
                        nc.tensor.matmul(pbc, ones_row[:, 0:64], rec_bf,
                                         start=True, stop=True)
                        rb_sb = pp.tile([64, LQ], BF16, tag="rb_sb", bufs=2,
                                        name=f"rbc{h}")
                        nc.vector.tensor_copy(rb_sb, pbc)
                        nc.vector.tensor_mul(att2[m // 2][rs, m % 2, :],
                                             po[0:64, :], rb_sb)
                    return run

                def ca_po(h, w):
                    def run():
                        px = pexps.pop((h, w))
                        nc.tensor.matmul(pos[h], vcb8[w][:, :, h, :], px,
                                         start=(w == 0), stop=(w == NWC - 1),
                                         perf_mode=DR)
                    return run

                NIT = H * NWC
                for g in range(NIT + 4):
                    while pending and pending[0][0] <= g:
                        pending.pop(0)[1]()
                    if g >= NIT:
                        continue
                    h, w = divmod(g, NWC)
                    hg = h // 4
                    rs = slice((h % 4) * 32, (h % 4) * 32 + 32)
                    if w == 0:
                        pos[h] = ps.tile([65, LQ], FP32, tag="pC", bufs=2,
                                         name=f"poc{h}")
                    psc = ps.tile([128, 2, LQ], FP32, tag="pQ", bufs=2,
                                  name=f"pscc{h}_{w}")
                    if h % 4 == 3:
                        rs = slice(64, 128)
                        qmov = qcz8[hg]
                    else:
                        qmov = qcT8[hg]
                    for jj in (0, 1):
                        t = 2 * w + jj
                        nc.tensor.matmul(psc[:, jj, :],
                                         kcT8[hg][rs, :, t * 128:(t + 1) * 128],
                                         qmov[rs, :, :],
                                         start=True, stop=True,
                                         perf_mode=DR)
                    pexp = pca.tile([128, 2, LQ], F8, tag="pexpC", bufs=4,
                                    name=f"pexpc{h}_{w}")
                    nc.scalar.activation(out=pexp, in_=psc, func=AF.Exp,
                                         scale=KSC)
                    pexps[(h, w)] = pexp
                    pending.append((g + CLAG, ca_po(h, w)))
                    if w == 0 and h % 2 == 0 and h // 2 + 4 < 8:
                        hgn, jn = divmod(h // 2 + 4, 2)
                        qc_unit(hgn, jn)
                    if w == NWC - 1:
                        pending.append((g + CLAG + 1, ca_epilogue(h)))
                        if h % 2 == 0 and 2 + h // 2 < 8:
                            pending.append((g + CLAG + 1, (lambda mg:
                                lambda: gu_fetch(mg))(2 + h // 2)))

                while pending:
                    pending.pop(0)[1]()

                mark("ca_out")
                # ca_out (needs ALL heads' att2) + norm3 ssq (lagged)
                pssq3 = ps.tile([1, LQ], FP32, tag="pC", bufs=2, name="pssq_n3")

                def n3_ssq(m):
                    xsq = pca.tile([128, LQ], BF16, tag="xsq2", bufs=2,
                                   name=f"xsq3_{m}")
                    nc.scalar.activation(out=xsq, in_=xres[:, m, :],
                                         func=AF.Square)
                    nc.tensor.matmul(pssq3, ones_col, xsq,
                                     start=(m == 0), stop=(m == CT - 1))

                for m in range(CT):
                    ca_out_unit(m)
                for m in range(CT):
                    n3_ssq(m)

                mark("mlp_norm")
                # norm3 + modulation -> bf16, then hi/lo fp8 split
                xmb = [pca.tile([128, LQ], BF16, tag=f"xm{k}", name=f"xmb{k}")
                       for k in range(CT)]
                xh2 = [pca.tile([128, 2, LQ], F8, tag=f"xh{j}", name=f"xh{j}")
                       for j in range(KP)]
                xl2 = [pca.tile([128, 2, LQ], F8, tag=f"xl{j}", name=f"xl{j}")
                       for j in range(KP)]
                x64 = [pca.tile([128, 2, LQ], F8, tag=f"x6{j}", name=f"x6{j}")
                       for j in range(KP)]
                rstd3 = pca.tile([1, LQ], FP32, tag="rstd", bufs=2,
                                 name="rstd_n3")
                nc.scalar.activation(out=rstd3, in_=pssq3, func=AF.Sqrt,
                                     bias=eps_c, scale=1.0 / C)
                rstd3_bf = pca.tile([1, LQ], BF16, tag="rstd_bf", bufs=2,
                                    name="rstdb_n3")
                with nc.allow_low_precision(reason="rstd bf16"):
                    nc.vector.reciprocal(rstd3_bf, rstd3)
                pb3 = ps.tile([128, LQ], FP32, tag="pC", bufs=2, name="pb_n3")
                nc.tensor.matmul(pb3, ones_row, rstd3_bf, start=True, stop=True)
                pb3sb = pca.tile([128, LQ], BF16, tag="pb2sb", bufs=1,
                                 name="pb3sb")
                nc.scalar.copy(out=pb3sb, in_=pb3)
                for k in range(CT):
                    nc.vector.scalar_tensor_tensor(
                        out=xmb[k], in0=xres[:, k, :], scalar=w3eff[:, k:k + 1],
                        in1=pb3sb,
                        op0=mybir.AluOpType.mult, op1=mybir.AluOpType.mult)
                    nc.gpsimd.tensor_scalar(out=xmb[k], in0=xmb[k],
                                            scalar1=sh_ml(k), scalar2=None,
                                            op0=mybir.AluOpType.add)
                    hi = xh2[k // 2][:, k % 2, :]
                    lo = xl2[k // 2][:, k % 2, :]
                    eh = nc.gpsimd if k % 2 == 0 else nc.vector
                    eh.tensor_copy(hi, xmb[k])
                    eh.tensor_sub(lo, xmb[k], hi)
                    nc.scalar.activation(out=x64[k // 2][:, k % 2, :],
                                         in_=xmb[k], func=AF.Identity,
                                         scale=1.0 / LOSC)

                mark("gate_up")
                # h2: fp8 DR-paired ffn activations
                h2 = [pca.tile([128, 2, LQ], F8, tag=f"h{t}", name=f"h2_{t}")
                      for t in range(FF // 256)]
                h64_2 = [pca.tile([128, 2, LQ], F8, tag=f"h6{t}", name=f"h64_{t}")
                         for t in range(FF // 256)]
                wdh_t, wdl_t = {}, {}

                def down_fetch(m):
                    wdh_t[m] = pca.tile([128, 16, 2, 128], F8, tag="wdw", bufs=4,
                                        name=f"wdh{m}")
                    dma(out=wdh_t[m], in_=wdh[m])
                    wdl_t[m] = pca.tile([128, 16, 2, 128], F8, tag="wdw", bufs=4,
                                        name=f"wdl{m}")
                    dma(out=wdl_t[m], in_=wdl[m])

                def dr_hilo(p1, wh, wl, mi, xlo=True):
                    # Xh*Wh + (X/64)*(Wl*64) [+ Xl*Wh], all at true scale
                    ms = slice(mi * 128, (mi + 1) * 128)
                    for kp in range(KP):
                        nc.tensor.matmul(p1, wh[:, kp, :, ms], xh2[kp][:, :, :],
                                         start=(kp == 0), stop=False, perf_mode=DR)
                    for kp in range(KP):
                        nc.tensor.matmul(p1, wl[:, kp, :, ms], x64[kp][:, :, :],
                                         start=False, stop=(not xlo and kp == KP - 1),
                                         perf_mode=DR)
                    if xlo:
                        for kp in range(KP):
                            nc.tensor.matmul(p1, wh[:, kp, :, ms], xl2[kp][:, :, :],
                                             start=False, stop=(kp == KP - 1),
                                             perf_mode=DR)

                for mg in range(8):
                    if mg >= 6:
                        down_fetch(mg - 6)
                    for mi in range(4):
                        pgu = ps.tile([128, 2 * LQ], FP32, tag="pQ", bufs=2,
                                      name=f"pgu{mg}_{mi}")
                        p1g = pgu[:, 0:LQ]
                        p1u = pgu[:, LQ:2 * LQ]
                        dr_hilo(p1g, wgh_t[mg], wgl_t[mg], mi)
                        sgl = pca.tile([128, LQ], BF16, tag="sgb", bufs=2,
                                       name=f"sgl{mg}_{mi}")
                        nc.scalar.activation(out=sgl, in_=p1g, func=AF.Silu)
                        dr_hilo(p1u, wuh_t[mg], wul_t[mg], mi)
                        t = mg * 4 + mi
                        nc.vector.tensor_mul(h2[t // 2][:, t % 2, :], sgl, p1u)
                        h64 = h64_2[t // 2][:, t % 2, :]
                        nc.scalar.activation(
                            out=h64, in_=h2[t // 2][:, t % 2, :],
                            func=AF.Identity, scale=1.0 / LOSC)

                mark("down")
                # down proj: P1 = H*Wdh + H64*Wdl64; out = P1*g + xres
                for m in range(CT):
                    if m + 2 < CT:
                        down_fetch(m + 2)
                    pd1 = ps.tile([128, LQ], FP32, tag="pA", bufs=2, name=f"pd1{m}")
                    for fp in range(16):
                        nc.tensor.matmul(pd1, wdh_t[m][:, fp, :, :],
                                         h2[fp][:, :, :],
                                         start=(fp == 0), stop=False,
                                         perf_mode=DR)
                    for fp in range(16):
                        nc.tensor.matmul(pd1, wdl_t[m][:, fp, :, :],
                                         h64_2[fp][:, :, :],
                                         start=False, stop=(fp == 15),
                                         perf_mode=DR)
                    of = pca.tile([128, LQ], FP32, tag="of", bufs=2, name=f"of{m}")
                    nc.vector.scalar_tensor_tensor(
                        out=of, in0=pd1, scalar=g_ml(m), in1=xres[:, m, :],
                        op0=mybir.AluOpType.mult, op1=mybir.AluOpType.add)
                    dma(out=outT[m * 128:(m + 1) * 128, :], in_=of)

    nc.compile()
    return nc


def _dr_perm(rope: bool):
    # column order for the (hg, j) DR layout: new col o=(hg*2+j)*128+p
    # takes original W column idx[o].
    idx = np.zeros(C, dtype=np.int64)
    for hg in range(4):
        for j in range(2):
            for p in range(128):
                h = hg * 4 + p // 32
                d = j * 32 + p % 32
                if rope:
                    # rotate-half pair i=(d%32): real=2i, imag=2i+1
                    c0 = h * 64 + 2 * (d % 32) + (0 if d < 32 else 1)
                else:
                    c0 = h * 64 + d
                idx[(hg * 2 + j) * 128 + p] = c0
    return idx


def _bf(a):
    return np.ascontiguousarray(a).astype(ml_dtypes.bfloat16)


def _f8(a):
    return np.ascontiguousarray(a).astype(F8NP)


def _dr_pack(W):
    # [n_in, n_out] -> [128, n_in//256, 2, n_out]
    n_in, n_out = W.shape
    kp = n_in // 256
    return W.reshape(kp, 2, 128, n_out).transpose(2, 0, 1, 3)


def _hilo(W):
    hi = W.astype(F8NP)
    lo = ((W - hi.astype(np.float32)) * LOSC).astype(F8NP)
    return hi, lo


def _prep_shared(W_qkv, W_sa_out, W_q, W_kv, W_ca_out, W_gate, W_up, W_down,
                 adaLN_W, adaLN_b, norm1_w, norm2_w, norm3_w):
    idx_r = _dr_perm(True)
    idx_n = _dr_perm(False)
    wq = W_qkv[:, 0:C][:, idx_r]
    wk = W_qkv[:, C:2 * C][:, idx_r]
    wv = W_qkv[:, 2 * C:3 * C]
    wqc = np.asarray(W_q, np.float32)[:, idx_n]
    wkv = np.concatenate([np.asarray(W_kv, np.float32)[:, 0:C][:, idx_n],
                          np.asarray(W_kv, np.float32)[:, C:2 * C]], axis=1)

    def pack8(W):
        return _f8(_dr_pack(np.asarray(W, np.float32)))

    wgh_, wgl_ = _hilo(np.asarray(W_gate, np.float32))
    wuh_, wul_ = _hilo(np.asarray(W_up, np.float32))
    wdh_, wdl_ = _hilo(np.asarray(W_down, np.float32))

    def mlp_pack(w8):  # fp8 [C, FF] -> [8 mg][128, kp, 2, 512]
        d = _dr_pack(w8.astype(np.float32)).astype(F8NP)
        return np.ascontiguousarray(d.reshape(128, KP, 2, 8, 512)
                                    .transpose(3, 0, 1, 2, 4))

    def down_pack(w8):  # fp8 [FF, C] -> [8 m][128, 16 fp, 2, 128]
        d = _dr_pack(w8.astype(np.float32)).astype(F8NP)
        return np.ascontiguousarray(d.reshape(128, 16, 2, CT, 128)
                                    .transpose(3, 0, 1, 2, 4))

    wada = np.asarray(adaLN_W, np.float32).reshape(CT, 128, 48, 128)
    wadaA_h = wada[:, :, 0:16, :].transpose(1, 0, 2, 3).reshape(128, CT, 2048)
    wadaB_h = np.stack([
        wada[:, :, 16 + 4 * i:20 + 4 * i, :].transpose(1, 0, 2, 3)
        .reshape(128, CT, 512) for i in range(8)])

    sh = {
        "wq2": pack8(wq), "wk2": pack8(wk), "wv2": pack8(wv),
        "wsa2": pack8(W_sa_out), "wqc2": pack8(wqc), "wkv2": pack8(wkv),
        "wca2": pack8(W_ca_out),
        "wgh": mlp_pack(wgh_), "wgl": mlp_pack(wgl_),
        "wuh": mlp_pack(wuh_), "wul": mlp_pack(wul_),
        "wdh": down_pack(wdh_), "wdl": down_pack(wdl_),
        "wadaA": _f8(wadaA_h), "wadaB": _bf(wadaB_h),
        "cst_base": np.concatenate([
            np.asarray(adaLN_b, np.float32).reshape(48, 128).T,
            np.asarray(norm1_w, np.float32).reshape(8, 128).T,
            np.asarray(norm2_w, np.float32).reshape(8, 128).T,
            np.asarray(norm3_w, np.float32).reshape(8, 128).T], axis=1),
    }
    return sh


def make_in_maps(x, t_mod, audio_context, freqs_cos, freqs_sin,
                 norm1_w, norm2_w, norm3_w,
                 W_qkv, W_sa_out, W_q, W_kv, W_ca_out,
                 W_gate, W_up, W_down, adaLN_W, adaLN_b):
    sh = _prep_shared(W_qkv, W_sa_out, W_q, W_kv, W_ca_out, W_gate, W_up,
                      W_down, adaLN_W, adaLN_b, norm1_w, norm2_w, norm3_w)
    cosT = np.ascontiguousarray(np.asarray(freqs_cos, np.float32).T)
    sinT = np.ascontiguousarray(np.asarray(freqs_sin, np.float32).T)

    in_maps = []
    for core in range(NCORE):
        b, j = divmod(core, 4)
        # roll the token axis so this core's own 512 tokens sit at [0, LQ)
        xT = np.roll(np.ascontiguousarray(np.asarray(x, np.float32)[b].T),
                     -j * LQ, axis=1)
        m = {k: v for k, v in sh.items() if k != "cst_base"}
        m["x_bf"] = _bf(xT)
        m["xq_f"] = np.ascontiguousarray(xT[:, 0:LQ])
        cr = np.roll(cosT, -j * LQ, axis=1)
        sr = np.roll(sinT, -j * LQ, axis=1)
        m["cs4"] = _bf(np.concatenate([cr, cr, cr, cr], axis=0))
        m["ss4"] = _bf(np.concatenate([sr, sr, sr, sr], axis=0))
        m["aud2"] = _f8(_dr_pack(
            np.ascontiguousarray(np.asarray(audio_context, np.float32)[b].T)))
        m["cst"] = np.ascontiguousarray(np.concatenate(
            [np.asarray(t_mod, np.float32)[b].reshape(8, 128).T,
             sh["cst_base"]], axis=1))
        in_maps.append(m)
    return in_maps


_NC_CACHE = None


def _get_nc():
    global _NC_CACHE
    if _NC_CACHE is None:
        _NC_CACHE = build_bass()
    return _NC_CACHE


def kernel(**inputs):
    nc = _get_nc()
    inputs = {k: np.asarray(v) for k, v in inputs.items()}
    in_maps = make_in_maps(**inputs)
    res = run_bass_kernel_spmd(nc, in_maps, list(range(NCORE)))
    out = np.zeros((B, L, C), np.float32)
    for core in range(NCORE):
        b, j = divmod(core, 4)
        out[b, j * LQ:(j + 1) * LQ, :] = res.results[core]["outT"].T
    return out
